# revision 29
# baseline (speedup 1.0000x reference)
"""Trainium2 Bass kernel for nn_Attention_51307679318359.

Multi-head attention (B=2, S=2048, D=2048, H=16, HD=128) with RoPE and an
additive mask, sharded over 8 NeuronCores as (batch x head-group): each core
computes 1 batch and 4 heads (512 channels), producing a partial output that
the host sums over head-groups.

v4 (bf16, SBUF-resident, pair-wide phase B):
- all operands bf16 (host-converted); Q/K/V and attn stay SBUF-resident.
- phase B processes sq in 1024-wide jq-PAIRS: one wide scores matmul, one
  wide exp, one wide AV and one wide denominator matmul per (pair, sk-tile)
  block -- half the instruction count of 512-wide blocks.  Software
  pipelined (lookahead 1 pair-block) so the PE never waits on ACT.
- softmax normalization via gpsimd partition_broadcast (no PE broadcast
  matmul); causal diagonal blocks are column-trimmed.
- phase A runs the first Q projection in k-waves so the PE starts as soon
  as the first 0.5MB of wq/x lands; all non-critical DMAs are gated behind
  early compute via tiny WAW corner-writes so they don't steal HBM
  bandwidth from the critical-path loads.
"""

import math

import numpy as np
import ml_dtypes

import concourse.bass as bass
import concourse.mybir as mybir
import concourse.tile as tile
from concourse import bacc
from concourse import bass_utils

F32 = mybir.dt.float32
F32R = mybir.dt.float32r
BF16 = mybir.dt.bfloat16
ADD = mybir.AluOpType.add
MULT = mybir.AluOpType.mult
COPY = mybir.ActivationFunctionType.Copy
EXP = mybir.ActivationFunctionType.Exp

B, S, D = 2, 2048, 2048
H, HD = 16, 128
NCORES = 8
GROUPS = NCORES // B          # 4 head-groups
HPG = H // GROUPS             # 4 heads per group
C = HPG * HD                  # 512 per-core channels
P = 128
CH_A = 512                    # phase-A s-chunk width
SQ = 512                      # mask-classification sq-chunk width
PW = 1024                     # phase-B pair window (2 * SQ)
SCALE = 1.0 / math.sqrt(HD)
NEG_THRESH = -1e8             # "masked out" threshold
BF = ml_dtypes.bfloat16

_PROGRAM_CACHE = {}


def _pre_w(wT):
    """(d, c) row-major -> (128, d//128, c) partition-major contiguous bf16."""
    d, c = wT.shape
    return np.ascontiguousarray(
        wT.reshape(d // P, P, c).transpose(1, 0, 2)).astype(BF)


def _pre_x(xT):
    """(d, s) -> (s//CH_A, 128, d//128, CH_A) chunk-major contiguous bf16."""
    d, s = xT.shape
    return np.ascontiguousarray(
        xT.reshape(d // P, P, s // CH_A, CH_A).transpose(2, 1, 0, 3)).astype(BF)


def _classify_mask(mask):
    """Classify transposed-mask blocks (sk-tile i x sq-chunk j).  Returns
    (classes, patterns): classes[(j, i)] is 'skip' | ('plain', qoff) |
    (pat_idx, qoff) where qoff counts leading fully-masked sq columns of the
    block (trimmed from all device matmuls).  patterns is (nblk, 128, SQ)
    bf16 holding exp(maskT block)."""
    maskT = np.ascontiguousarray(mask.T)
    n_j = mask.shape[0] // SQ
    n_i = mask.shape[0] // P
    classes = {}
    patterns = []
    pat_idx = {}

    def add_pattern(blk):
        key = blk.tobytes()
        if key not in pat_idx:
            pat_idx[key] = len(patterns)
            with np.errstate(over='ignore'):
                patterns.append(np.exp(blk.astype(np.float64)).astype(BF))
        return pat_idx[key]

    for j in range(n_j):
        for i in range(n_i):
            blk = maskT[i * P:(i + 1) * P, j * SQ:(j + 1) * SQ]
            dead_col = np.all(blk <= NEG_THRESH, axis=0)  # (SQ,)
            if np.all(dead_col):
                classes[(j, i)] = 'skip'
                continue
            qoff = 0
            while dead_col[qoff]:
                qoff += 1
            rest = blk[:, qoff:]
            if np.all(rest == 0.0):
                classes[(j, i)] = ('plain', qoff)
            else:
                classes[(j, i)] = (add_pattern(blk), qoff)
    # every sq position must keep at least one live sk tile, else softmax
    # denominators vanish; fall back to no skipping in that degenerate case
    if any(all(classes[(j, i)] == 'skip' for i in range(n_i)) for j in range(n_j)):
        for j in range(n_j):
            for i in range(n_i):
                if classes[(j, i)] == 'skip':
                    blk = maskT[i * P:(i + 1) * P, j * SQ:(j + 1) * SQ]
                    classes[(j, i)] = (add_pattern(blk), 0)
    # the first live block of each sq chunk must cover its full width (it
    # initializes the PSUM accumulation); widen it to qoff=0, which requires
    # the pattern (leading columns are fully masked -> exp(mask)=0 there)
    for j in range(n_j):
        for i in range(n_i):
            cls = classes[(j, i)]
            if cls == 'skip':
                continue
            if cls[1] != 0:
                blk = maskT[i * P:(i + 1) * P, j * SQ:(j + 1) * SQ]
                classes[(j, i)] = (add_pattern(blk), 0)
            break
    pats = np.stack(patterns, 0).astype(BF) if patterns else \
        np.zeros((1, P, SQ), BF)
    return classes, pats


def _pair_blocks(classes, n_j, n_i):
    """Group sq chunks into pairs and build per-(pair, sk-tile) blocks.

    Returns pairs: list of (pair_idx, base_jq, width_chunks, blocks) where
    blocks is a list of dicts with:
      i, segs: [(cs, ce, start_flag)], pats: [(off, qoff, cls)]
    cs/ce are columns relative to the pair window.  A single contiguous
    segment is merged across both halves when possible."""
    pairs = []
    pj = 0
    pidx = 0
    while pj < n_j:
        wchunks = 2 if pj + 1 < n_j else 1
        jqs = list(range(pj, pj + wchunks))
        half_first = {jq: True for jq in jqs}
        blocks = []
        for i in range(n_i):
            halves = []
            for hh, jq in enumerate(jqs):
                cls = classes[(jq, i)]
                if cls == 'skip':
                    continue
                pat, qoff = cls
                halves.append((hh, pat, qoff))
            if not halves:
                continue
            segs = []
            pats_ = []
            firsts = [half_first[jqs[h[0]]] for h in halves]
            merged = (len(halves) == 2 and halves[1][2] == 0
                      and firsts[0] == firsts[1])
            if merged:
                hh0, pat0, qoff0 = halves[0]
                segs.append((qoff0, 2 * SQ, firsts[0]))
            else:
                for hh, pat, qoff in halves:
                    segs.append((hh * SQ + qoff, (hh + 1) * SQ,
                                 half_first[jqs[hh]]))
            for hh, pat, qoff in halves:
                if isinstance(pat, int):
                    pats_.append((hh * SQ, qoff, pat))
                half_first[jqs[hh]] = False
            blocks.append(dict(i=i, segs=segs, pats=pats_))
        # last block that writes any column < SQ (the lo half): its copies
        # can be emitted early, overlapping the remaining hi-only blocks
        lo_last = -1
        if wchunks == 2:
            for bi, blk in enumerate(blocks):
                if any(cs < SQ for (cs, ce, sf) in blk['segs']):
                    lo_last = bi
            if lo_last == len(blocks) - 1:
                lo_last = -1
        # pair up adjacent blocks with identical segment coverage: their
        # probs are summed on the DVE and contribute a single denominator
        # matmul (dnrole: 0 = solo, 1 = defer into next, 2 = paired-with-prev)
        bi = 0
        while bi < len(blocks):
            b0 = blocks[bi]
            if (bi + 1 < len(blocks)
                    and blocks[bi + 1]['segs'] == b0['segs']
                    and all(not sf for (_, _, sf) in blocks[bi + 1]['segs'])):
                b0['dnrole'] = 1
                blocks[bi + 1]['dnrole'] = 2
                bi += 2
            else:
                b0['dnrole'] = 0
                bi += 1
        pairs.append((pidx, pj, wchunks, blocks, lo_last))
        pj += wchunks
        pidx += 1
    return pairs


def _build(classes, nblk, s=S, d=D):
    """Build + compile the per-core SPMD program."""
    nkt = d // P
    n_j = s // SQ
    n_i = s // P
    n_ja = s // CH_A

    nc = bacc.Bacc("TRN2", target_bir_lowering=False, debug=False)
    xT = nc.dram_tensor("xT", (n_ja, P, nkt, CH_A), BF16, kind="ExternalInput")
    wqT = nc.dram_tensor("wqT", (P, nkt, C), BF16, kind="ExternalInput")
    wkT = nc.dram_tensor("wkT", (P, nkt, C), BF16, kind="ExternalInput")
    wvT = nc.dram_tensor("wvT", (P, nkt, C), BF16, kind="ExternalInput")
    woT = nc.dram_tensor("woT", (P, HPG, d), BF16, kind="ExternalInput")
    cosP = nc.dram_tensor("cosP", (HD, s), BF16, kind="ExternalInput")
    sinSw = nc.dram_tensor("sinSw", (HD, s), BF16, kind="ExternalInput")
    mblk = nc.dram_tensor("mblk", (P, nblk, SQ), BF16, kind="ExternalInput")
    onesb = nc.dram_tensor("onesb", (P, 1), BF16, kind="ExternalInput")
    out = nc.dram_tensor("out", (s, d), BF16, kind="ExternalOutput")

    pairs = _pair_blocks(classes, n_j, n_i)

    with tile.TileContext(nc) as tc:
        with tc.tile_pool(name="const", bufs=1) as const:
            cos_t = const.tile([P, s], BF16)
            sin_t = const.tile([P, s], BF16)
            ones_r = const.tile([P, 1], BF16)
            nc.gpsimd.dma_start(ones_r[:], onesb[:])
            mblk_t = const.tile([P, nblk, SQ], BF16)
            qh_sb = const.tile([P, HPG, s], BF16)
            kh_sb = const.tile([P, HPG, s], BF16)
            vh_sb = const.tile([P, n_i, C], BF16)
            attn_t = const.tile([P, HPG, s], BF16)
            wo_t = const.tile([P, HPG, d], BF16)

            # ---------------- Phase A: QKV projections + RoPE ----------------
            with tc.tile_pool(name="wres", bufs=1) as wres, \
                 tc.tile_pool(name="xc", bufs=2) as xcp, \
                 tc.tile_pool(name="ptmp", bufs=4) as ptmp, \
                 tc.tile_pool(name="psW", bufs=HPG, space="PSUM") as psW, \
                 tc.tile_pool(name="psA", bufs=2, space="PSUM") as psA:
                wq_t = wres.tile([P, nkt, C], BF16)
                wk_t = wres.tile([P, nkt, C], BF16)
                wv_t = wres.tile([P, nkt, C], BF16)
                qk4 = nkt // 4
                qk8 = nkt // 8
                # critical-path loads, k-chunked: wq + x chunk 0 only.
                for q8 in range(8):
                    ksl = slice(q8 * qk8, (q8 + 1) * qk8)
                    nc.sync.dma_start(wq_t[:, ksl, :], wqT[:, ksl, :])
                xc_tiles = {}

                def get_xc(j):
                    if j not in xc_tiles:
                        xc_tiles[j] = xcp.tile([P, nkt, CH_A], BF16, tag="xc",
                                               name=f"xc{j}")
                    return xc_tiles[j]

                xc0 = get_xc(0)
                for q8 in range(8):
                    ksl = slice(q8 * qk8, (q8 + 1) * qk8)
                    nc.scalar.dma_start(xc0[:, ksl, :], xT[0][:, ksl, :])
                # HAM warmup: tiny matmuls keep the PE activity monitor busy
                # during the initial DMA wait so the real stream starts at
                # the full 2.4GHz clock instead of the cold 1.2GHz p-state
                warm = psA.tile([P, CH_A], F32, tag="ps_qk", name="warm")
                for _ in range(40):
                    nc.tensor.matmul(warm[0:1, 0:1], ones_r[:], ones_r[:],
                                     start=True, stop=True,
                                     skip_group_check=True)
                # gate wk/cos/sin on the last x0 chunk's arrival
                for q4 in range(4):
                    nc.vector.tensor_copy(wk_t[0:1, q4 * qk4, 0:1],
                                          xc0[0:1, nkt - 1, CH_A - 1:CH_A])
                nc.vector.tensor_copy(cos_t[0:1, 0:1],
                                      xc0[0:1, nkt - 1, CH_A - 1:CH_A])
                nc.vector.tensor_copy(sin_t[0:1, 0:1],
                                      xc0[0:1, nkt - 1, CH_A - 1:CH_A])
                for q4 in range(4):
                    ksl = slice(q4 * qk4, (q4 + 1) * qk4)
                    nc.gpsimd.dma_start(wk_t[:, ksl, :], wkT[:, ksl, :])
                nc.sync.dma_start(cos_t[:], cosP[:])
                nc.sync.dma_start(sin_t[:], sinSw[:])

                def rope(ps, dst, ct, sl, j, wt):
                    # RoPE (rotate-half layout):
                    #   out_top = x0*cos - x1*sin ; out_bot = x1*cos + x0*sin
                    # ACT stages PSUM->SBUF bf16 so DVE gets 2x mode
                    pc = ptmp.tile([P, CH_A], BF16, tag="pc")
                    nc.scalar.activation(pc[:], ps[:], COPY)
                    if j == 0 and wt is wq_t and ct == 0:
                        # release wv + x chunk 1 loads (needed ~25us in)
                        for q4 in range(4):
                            nc.vector.tensor_copy(
                                wv_t[0:1, q4 * qk4, 0:1], pc[0:1, 0:1])
                        for q4 in range(4):
                            ksl = slice(q4 * qk4, (q4 + 1) * qk4)
                            nc.gpsimd.dma_start(wv_t[:, ksl, :], wvT[:, ksl, :])
                    if j == 0 and wt is wq_t and ct == 1:
                        xc1 = get_xc(1)
                        nc.vector.tensor_copy(xc1[0:1, 0, 0:1], pc[0:1, 0:1])
                        nc.scalar.dma_start(xc1[:], xT[1])
                    t1 = ptmp.tile([P, CH_A], BF16, tag="t1")
                    nc.vector.tensor_tensor(t1[:], pc[:], cos_t[:, sl], MULT)
                    t2 = ptmp.tile([P, CH_A], BF16, tag="t2")
                    nc.vector.tensor_tensor(
                        t2[0:64, :], pc[64:128, :], sin_t[64:128, sl], MULT)
                    nc.vector.tensor_tensor(
                        t2[64:128, :], pc[0:64, :], sin_t[0:64, sl], MULT)
                    nc.vector.tensor_tensor(dst[:, ct, sl], t1[:], t2[:], ADD)

                for j in range(n_ja):
                    sl = slice(j * CH_A, (j + 1) * CH_A)
                    xc = get_xc(j)
                    if j > 1:
                        nc.scalar.dma_start(xc[:], xT[j])
                    if j == 0:
                        # Q projection in k-waves: wave kc needs only the
                        # kc-th wq/x k-chunk, so the PE starts on the first
                        # 0.5MB of each instead of waiting for the full 2MB.
                        pss = [psW.tile([P, CH_A], F32, tag="ps_w",
                                        name=f"psw{ct}") for ct in range(HPG)]
                        for kc in range(8):
                            for ct in range(HPG):
                                for k in range(kc * qk8, (kc + 1) * qk8):
                                    nc.tensor.matmul(
                                        pss[ct][:],
                                        wq_t[:, k, ct * P:(ct + 1) * P],
                                        xc[:, k, :],
                                        start=(k == 0), stop=(k == nkt - 1),
                                        skip_group_check=True)
                        for ct in range(HPG):
                            rope(pss[ct], qh_sb, ct, sl, j, wq_t)
                        qk_iter = ((wk_t, kh_sb),)
                    else:
                        qk_iter = ((wq_t, qh_sb), (wk_t, kh_sb))
                    for (wt, dst) in qk_iter:
                        for ct in range(HPG):
                            ps = psA.tile([P, CH_A], F32, tag="ps_qk")
                            for k in range(nkt):
                                nc.tensor.matmul(
                                    ps[:], wt[:, k, ct * P:(ct + 1) * P],
                                    xc[:, k, :],
                                    start=(k == 0), stop=(k == nkt - 1))
                            rope(ps, dst, ct, sl, j, wt)
                    for st2 in range(CH_A // P):
                        st = (j * CH_A) // P + st2
                        psv = psA.tile([P, C], F32, tag="ps_v")
                        for k in range(nkt):
                            nc.tensor.matmul(
                                psv[:], xc[:, k, st2 * P:(st2 + 1) * P],
                                wv_t[:, k, :],
                                start=(k == 0), stop=(k == nkt - 1))
                        nc.scalar.activation(vh_sb[:, st, :], psv[:], COPY)
                        if j == 0 and st2 == 0:
                            # release the late constant loads (wo/mblk)
                            nc.vector.tensor_copy(
                                wo_t[0:1, 0, 0:1], vh_sb[0:1, 0, 0:1])
                            nc.vector.tensor_copy(
                                mblk_t[0:1, 0, 0:1], vh_sb[0:1, 0, 0:1])
                            nc.gpsimd.dma_start(wo_t[:], woT[:])
                            nc.gpsimd.dma_start(mblk_t[:], mblk[:])

            # ---------------- Phase B: attention per head ----------------
            # flat pair-block list, software-pipelined with lookahead 1:
            # PE stream is ... sc(n+1), av(n), dn(n) ... so exp(n) on ACT
            # overlaps av(n-1)/dn(n-1)/sc(n+1) on the PE.
            flat = []
            for h in range(HPG):
                for (pidx, base_jq, wchunks, blocks, lo_last) in pairs:
                    nb = len(blocks)
                    for bi, blk in enumerate(blocks):
                        flat.append((h, pidx, base_jq, wchunks, blk,
                                     bi == 0, bi == nb - 1, bi == lo_last))

            with tc.tile_pool(name="pr", bufs=4) as prp, \
                 tc.tile_pool(name="sm", bufs=2) as smp, \
                 tc.tile_pool(name="psS", bufs=2, space="PSUM") as psS, \
                 tc.tile_pool(name="psB", bufs=1, space="PSUM") as psB:
                prs = {}
                gps = {}
                has_lo = set()
                dnpend = {}

                def pieces(cs, ce):
                    # matmul moving operand caps at 512 and PSUM writes must
                    # stay within a bank: split segments at the SQ boundary
                    out = []
                    while cs < ce:
                        pe_ = min(ce, (cs // SQ + 1) * SQ)
                        out.append((cs, pe_))
                        cs = pe_
                    return out

                def emit_score(idx):
                    h, pidx, bjq, wch, blk, first, last_, lo_l = flat[idx]
                    w = wch * SQ
                    sc = psS.tile([P, PW], F32, tag="sc")
                    pr = prp.tile([P, PW], BF16, tag="pr")
                    base = bjq * SQ
                    for (cs, ce, sflag) in blk['segs']:
                        for (ps_, pe_) in pieces(cs, ce):
                            nc.tensor.matmul(
                                sc[:, ps_:pe_],
                                kh_sb[:, h, blk['i'] * P:(blk['i'] + 1) * P],
                                qh_sb[:, h, base + ps_:base + pe_],
                                start=True, stop=True)
                        nc.scalar.activation(
                            pr[:, cs:ce], sc[:, cs:ce], EXP, scale=SCALE)
                    for (off, qoff, cls) in blk['pats']:
                        nc.vector.tensor_tensor(
                            pr[:, off + qoff:off + SQ],
                            pr[:, off + qoff:off + SQ],
                            mblk_t[:, cls, qoff:], MULT)
                    prs[idx] = pr

                def emit_accum(idx):
                    h, pidx, bjq, wch, blk, first, last_, lo_l = flat[idx]
                    w = wch * SQ
                    if first:
                        gps[(h, pidx)] = (
                            psB.tile([P, PW], F32, tag="at",
                                     name=f"at{h}_{pidx}"),
                            psB.tile([1, PW], F32, tag="dn",
                                     name=f"dn{h}_{pidx}"),
                            smp.tile([1, PW], F32, tag="dnsb",
                                     name=f"dnsb{h}_{pidx}"))
                    at_ps, dn_ps, dn_sb = gps[(h, pidx)]
                    pr = prs.pop(idx)
                    i = blk['i']
                    for (cs, ce, sflag) in blk['segs']:
                        for (ps_, pe_) in pieces(cs, ce):
                            nc.tensor.matmul(
                                at_ps[:, ps_:pe_],
                                vh_sb[:, i, h * HD:(h + 1) * HD],
                                pr[:, ps_:pe_],
                                start=sflag, stop=last_, skip_group_check=True)
                    role = blk.get('dnrole', 0)
                    if role == 1:
                        # defer: this block's probs are summed with the next
                        # block's (identical coverage) for one shared dn MM
                        dnpend[(h, pidx)] = (pr, blk['segs'])
                    else:
                        dn_src = pr
                        dn_segs = blk['segs']
                        if role == 2:
                            # start flags come from the earlier block (it may
                            # be the group's initializer)
                            pr0, dn_segs = dnpend.pop((h, pidx))
                            dn_src = prp.tile([P, PW], BF16, tag="prsum")
                            for (cs, ce, sflag) in blk['segs']:
                                nc.vector.tensor_tensor(
                                    dn_src[:, cs:ce], pr0[:, cs:ce],
                                    pr[:, cs:ce], ADD)
                        for (cs, ce, sflag) in dn_segs:
                            for (ps_, pe_) in pieces(cs, ce):
                                nc.tensor.matmul(
                                    dn_ps[:, ps_:pe_], ones_r[:],
                                    dn_src[:, ps_:pe_],
                                    start=sflag, stop=last_,
                                    skip_group_check=True)
                    if lo_l and not last_:
                        # the lo half is final: copy it out now, overlapped
                        # with the remaining hi-only blocks, so the PSUM
                        # buffers recycle fast at the group boundary
                        nc.vector.tensor_copy(
                            attn_t[:, h, bjq * SQ:(bjq + 1) * SQ],
                            at_ps[:, 0:SQ])
                        nc.scalar.activation(dn_sb[:, 0:SQ], dn_ps[:, 0:SQ],
                                             COPY)
                        has_lo.add((h, pidx))
                    if last_:
                        qsl = slice(bjq * SQ, bjq * SQ + w)
                        # hi-half (or full) copies, hoisted ahead of the next
                        # group's queue entries so the buffers free ASAP
                        hi0 = SQ if (h, pidx) in has_lo else 0
                        nc.vector.tensor_copy(
                            attn_t[:, h, bjq * SQ + hi0:bjq * SQ + w],
                            at_ps[:, hi0:w])
                        nc.scalar.activation(dn_sb[:, hi0:w],
                                             dn_ps[:, hi0:w], COPY)
                        # fold + invert denominators; gpsimd broadcast; DVE
                        # normalize (deferred -- not on the PE critical path)
                        nf = w // P
                        dn4 = smp.tile([PW // P, P], F32, tag="dn4")
                        nc.gpsimd.dma_start(dn4[0:nf, :], dn_sb[:, 0:w])
                        rc4 = smp.tile([PW // P, P], F32, tag="rc4")
                        nc.vector.reciprocal(rc4[0:nf, :], dn4[0:nf, :])
                        rc4b = smp.tile([PW // P, P], BF16, tag="rc4b")
                        nc.vector.tensor_copy(rc4b[0:nf, :], rc4[0:nf, :])
                        rc = smp.tile([1, PW], BF16, tag="rc")
                        nc.gpsimd.dma_start(rc[:, 0:w], rc4b[0:nf, :])
                        bc_sb = smp.tile([P, PW], BF16, tag="bcs")
                        nc.gpsimd.partition_broadcast(bc_sb[:, 0:w],
                                                      rc[:, 0:w])
                        nc.vector.tensor_tensor(
                            attn_t[:, h, qsl], attn_t[:, h, qsl],
                            bc_sb[:, 0:w], MULT)

                LA = 1
                nfl = len(flat)
                for idx in range(nfl):
                    emit_score(idx)
                    if idx - LA >= 0:
                        emit_accum(idx - LA)
                for idx in range(nfl - LA, nfl):
                    emit_accum(idx)

            # ---------------- Phase C: output projection ----------------
            with tc.tile_pool(name="og", bufs=2) as ogp, \
                 tc.tile_pool(name="psC", bufs=4, space="PSUM") as psC:
                for st in range(n_i):
                    og = ogp.tile([P, d], BF16, tag="og")
                    for dch in range(d // SQ):
                        po = psC.tile([P, SQ], F32, tag="po")
                        for ct in range(HPG):
                            nc.tensor.matmul(
                                po[:], attn_t[:, ct, st * P:(st + 1) * P],
                                wo_t[:, ct, dch * SQ:(dch + 1) * SQ],
                                start=(ct == 0), stop=(ct == HPG - 1))
                        nc.scalar.activation(
                            og[:, dch * SQ:(dch + 1) * SQ], po[:], COPY)
                        nc.sync.dma_start(
                            out[st * P:(st + 1) * P, dch * SQ:(dch + 1) * SQ],
                            og[:, dch * SQ:(dch + 1) * SQ])

    nc.compile()
    return nc


def _class_key(classes):
    def k(v):
        return v if isinstance(v, str) else tuple(v)
    return tuple(sorted((jk, k(v)) for jk, v in classes.items()))


def _prep_host(inputs):
    """Shard + transpose + bf16-convert the full inputs into 8 per-core maps."""
    x = np.asarray(inputs["x"], np.float32)
    wq = np.asarray(inputs["wq"], np.float32)
    wk = np.asarray(inputs["wk"], np.float32)
    wv = np.asarray(inputs["wv"], np.float32)
    wo = np.asarray(inputs["wo"], np.float32)
    cos = np.asarray(inputs["cos"], np.float32)
    sin = np.asarray(inputs["sin"], np.float32)
    mask = np.asarray(inputs["mask"], np.float32)
    start_p = int(inputs["start_p"])

    s = x.shape[1]
    cos_u = cos[start_p:start_p + s]          # (s, HD/2)
    sin_u = sin[start_p:start_p + s]

    # rotate-half channel permutation within each head: [evens, odds]
    perm = np.concatenate(
        [h * HD + np.concatenate([np.arange(0, HD, 2), np.arange(1, HD, 2)])
         for h in range(H)])

    cosP = np.ascontiguousarray(
        np.concatenate([cos_u.T, cos_u.T], axis=0)).astype(BF)     # (128, s)
    sinSw = np.ascontiguousarray(
        np.concatenate([sin_u.T, -sin_u.T], axis=0)).astype(BF)    # (128, s)

    classes, pats = _classify_mask(mask)
    onesb = np.ones((P, 1), BF)

    in_maps = []
    for b in range(B):
        xTp = _pre_x(np.ascontiguousarray(x[b].T))
        for g in range(GROUPS):
            rows = perm[g * C:(g + 1) * C]
            in_maps.append({
                "xT": xTp,
                "wqT": _pre_w(wq[rows, :].T),
                "wkT": _pre_w(wk[rows, :].T),
                "wvT": _pre_w(wv[g * C:(g + 1) * C, :].T),
                "woT": _pre_w(wo[:, g * C:(g + 1) * C].T),
                "cosP": cosP,
                "sinSw": sinSw,
                "mblk": np.ascontiguousarray(pats.transpose(1, 0, 2)),
                "onesb": onesb,
            })
    return in_maps, classes, pats


def _run(inputs, trace=False):
    in_maps, classes, pats = _prep_host(inputs)
    key = (pats.shape[0], _class_key(classes))
    if key not in _PROGRAM_CACHE:
        _PROGRAM_CACHE[key] = _build(classes, pats.shape[0])
    nc = _PROGRAM_CACHE[key]
    res = bass_utils.run_bass_kernel_spmd(
        nc, in_maps, core_ids=list(range(NCORES)), trace=trace)
    out = np.zeros((B, S, D), np.float32)
    for b in range(B):
        acc = res.results[b * GROUPS]["out"].astype(np.float32).copy()
        for g in range(1, GROUPS):
            acc += res.results[b * GROUPS + g]["out"].astype(np.float32)
        out[b] = acc
    return out, res


def kernel(**inputs):
    out, _ = _run(inputs, trace=False)
    return out


# revision 32
# speedup vs baseline: 1.0053x; 1.0053x over previous
"""Trainium2 Bass kernel for nn_Attention_51307679318359.

Multi-head attention (B=2, S=2048, D=2048, H=16, HD=128) with RoPE and an
additive mask, sharded over 8 NeuronCores as (batch x head-group): each core
computes 1 batch and 4 heads (512 channels), producing a partial output that
the host sums over head-groups.

v4 (bf16, SBUF-resident, pair-wide phase B):
- all operands bf16 (host-converted); Q/K/V and attn stay SBUF-resident.
- phase B processes sq in 1024-wide jq-PAIRS: one wide scores matmul, one
  wide exp, one wide AV and one wide denominator matmul per (pair, sk-tile)
  block -- half the instruction count of 512-wide blocks.  Software
  pipelined (lookahead 1 pair-block) so the PE never waits on ACT.
- softmax normalization via gpsimd partition_broadcast (no PE broadcast
  matmul); causal diagonal blocks are column-trimmed.
- phase A runs the first Q projection in k-waves so the PE starts as soon
  as the first 0.5MB of wq/x lands; all non-critical DMAs are gated behind
  early compute via tiny WAW corner-writes so they don't steal HBM
  bandwidth from the critical-path loads.
"""

import math

import numpy as np
import ml_dtypes

import concourse.bass as bass
import concourse.mybir as mybir
import concourse.tile as tile
from concourse import bacc
from concourse import bass_utils

F32 = mybir.dt.float32
F32R = mybir.dt.float32r
BF16 = mybir.dt.bfloat16
ADD = mybir.AluOpType.add
MULT = mybir.AluOpType.mult
COPY = mybir.ActivationFunctionType.Copy
EXP = mybir.ActivationFunctionType.Exp

B, S, D = 2, 2048, 2048
H, HD = 16, 128
NCORES = 8
GROUPS = NCORES // B          # 4 head-groups
HPG = H // GROUPS             # 4 heads per group
C = HPG * HD                  # 512 per-core channels
P = 128
CH_A = 512                    # phase-A s-chunk width
SQ = 512                      # mask-classification sq-chunk width
PW = 1024                     # phase-B pair window (2 * SQ)
SCALE = 1.0 / math.sqrt(HD)
NEG_THRESH = -1e8             # "masked out" threshold
BF = ml_dtypes.bfloat16

_PROGRAM_CACHE = {}


def _pre_w(wT):
    """(d, c) row-major -> (128, d//128, c) partition-major contiguous bf16."""
    d, c = wT.shape
    return np.ascontiguousarray(
        wT.reshape(d // P, P, c).transpose(1, 0, 2)).astype(BF)


def _pre_x(xT):
    """(d, s) -> (s//CH_A, 128, d//128, CH_A) chunk-major contiguous bf16."""
    d, s = xT.shape
    return np.ascontiguousarray(
        xT.reshape(d // P, P, s // CH_A, CH_A).transpose(2, 1, 0, 3)).astype(BF)


def _classify_mask(mask):
    """Classify transposed-mask blocks (sk-tile i x sq-chunk j).  Returns
    (classes, patterns): classes[(j, i)] is 'skip' | ('plain', qoff) |
    (pat_idx, qoff) where qoff counts leading fully-masked sq columns of the
    block (trimmed from all device matmuls).  patterns is (nblk, 128, SQ)
    bf16 holding exp(maskT block)."""
    maskT = np.ascontiguousarray(mask.T)
    n_j = mask.shape[0] // SQ
    n_i = mask.shape[0] // P
    classes = {}
    patterns = []
    pat_idx = {}

    def add_pattern(blk):
        key = blk.tobytes()
        if key not in pat_idx:
            pat_idx[key] = len(patterns)
            with np.errstate(over='ignore'):
                patterns.append(np.exp(blk.astype(np.float64)).astype(BF))
        return pat_idx[key]

    for j in range(n_j):
        for i in range(n_i):
            blk = maskT[i * P:(i + 1) * P, j * SQ:(j + 1) * SQ]
            dead_col = np.all(blk <= NEG_THRESH, axis=0)  # (SQ,)
            if np.all(dead_col):
                classes[(j, i)] = 'skip'
                continue
            qoff = 0
            while dead_col[qoff]:
                qoff += 1
            rest = blk[:, qoff:]
            if np.all(rest == 0.0):
                classes[(j, i)] = ('plain', qoff)
            else:
                classes[(j, i)] = (add_pattern(blk), qoff)
    # every sq position must keep at least one live sk tile, else softmax
    # denominators vanish; fall back to no skipping in that degenerate case
    if any(all(classes[(j, i)] == 'skip' for i in range(n_i)) for j in range(n_j)):
        for j in range(n_j):
            for i in range(n_i):
                if classes[(j, i)] == 'skip':
                    blk = maskT[i * P:(i + 1) * P, j * SQ:(j + 1) * SQ]
                    classes[(j, i)] = (add_pattern(blk), 0)
    # the first live block of each sq chunk must cover its full width (it
    # initializes the PSUM accumulation); widen it to qoff=0, which requires
    # the pattern (leading columns are fully masked -> exp(mask)=0 there)
    for j in range(n_j):
        for i in range(n_i):
            cls = classes[(j, i)]
            if cls == 'skip':
                continue
            if cls[1] != 0:
                blk = maskT[i * P:(i + 1) * P, j * SQ:(j + 1) * SQ]
                classes[(j, i)] = (add_pattern(blk), 0)
            break
    pats = np.stack(patterns, 0).astype(BF) if patterns else \
        np.zeros((1, P, SQ), BF)
    return classes, pats


def _pair_blocks(classes, n_j, n_i):
    """Group sq chunks into pairs and build per-(pair, sk-tile) blocks.

    Returns pairs: list of (pair_idx, base_jq, width_chunks, blocks) where
    blocks is a list of dicts with:
      i, segs: [(cs, ce, start_flag)], pats: [(off, qoff, cls)]
    cs/ce are columns relative to the pair window.  A single contiguous
    segment is merged across both halves when possible."""
    pairs = []
    pj = 0
    pidx = 0
    while pj < n_j:
        wchunks = 2 if pj + 1 < n_j else 1
        jqs = list(range(pj, pj + wchunks))
        half_first = {jq: True for jq in jqs}
        blocks = []
        for i in range(n_i):
            halves = []
            for hh, jq in enumerate(jqs):
                cls = classes[(jq, i)]
                if cls == 'skip':
                    continue
                pat, qoff = cls
                halves.append((hh, pat, qoff))
            if not halves:
                continue
            segs = []
            pats_ = []
            firsts = [half_first[jqs[h[0]]] for h in halves]
            merged = (len(halves) == 2 and halves[1][2] == 0
                      and firsts[0] == firsts[1])
            if merged:
                hh0, pat0, qoff0 = halves[0]
                segs.append((qoff0, 2 * SQ, firsts[0]))
            else:
                for hh, pat, qoff in halves:
                    segs.append((hh * SQ + qoff, (hh + 1) * SQ,
                                 half_first[jqs[hh]]))
            for hh, pat, qoff in halves:
                if isinstance(pat, int):
                    pats_.append((hh * SQ, qoff, pat))
                half_first[jqs[hh]] = False
            blocks.append(dict(i=i, segs=segs, pats=pats_))
        # last block that writes any column < SQ (the lo half): its copies
        # can be emitted early, overlapping the remaining hi-only blocks
        lo_last = -1
        if wchunks == 2:
            for bi, blk in enumerate(blocks):
                if any(cs < SQ for (cs, ce, sf) in blk['segs']):
                    lo_last = bi
            if lo_last == len(blocks) - 1:
                lo_last = -1
        # pair up adjacent blocks with identical segment coverage: their
        # probs are summed on the DVE and contribute a single denominator
        # matmul (dnrole: 0 = solo, 1 = defer into next, 2 = paired-with-prev)
        bi = 0
        while bi < len(blocks):
            b0 = blocks[bi]
            if (bi + 1 < len(blocks)
                    and blocks[bi + 1]['segs'] == b0['segs']
                    and all(not sf for (_, _, sf) in blocks[bi + 1]['segs'])):
                b0['dnrole'] = 1
                blocks[bi + 1]['dnrole'] = 2
                bi += 2
            else:
                b0['dnrole'] = 0
                bi += 1
        pairs.append((pidx, pj, wchunks, blocks, lo_last))
        pj += wchunks
        pidx += 1
    return pairs


def _build(classes, nblk, s=S, d=D):
    """Build + compile the per-core SPMD program."""
    nkt = d // P
    n_j = s // SQ
    n_i = s // P
    n_ja = s // CH_A

    nc = bacc.Bacc("TRN2", target_bir_lowering=False, debug=False)
    xT = nc.dram_tensor("xT", (n_ja, P, nkt, CH_A), BF16, kind="ExternalInput")
    wqT = nc.dram_tensor("wqT", (P, nkt, C), BF16, kind="ExternalInput")
    wkT = nc.dram_tensor("wkT", (P, nkt, C), BF16, kind="ExternalInput")
    wvT = nc.dram_tensor("wvT", (P, nkt, C), BF16, kind="ExternalInput")
    woT = nc.dram_tensor("woT", (P, HPG, d), BF16, kind="ExternalInput")
    cosP = nc.dram_tensor("cosP", (HD, s), BF16, kind="ExternalInput")
    sinSw = nc.dram_tensor("sinSw", (HD, s), BF16, kind="ExternalInput")
    mblk = nc.dram_tensor("mblk", (P, nblk, SQ), BF16, kind="ExternalInput")
    onesb = nc.dram_tensor("onesb", (P, 1), BF16, kind="ExternalInput")
    out = nc.dram_tensor("out", (s, d), BF16, kind="ExternalOutput")

    pairs = _pair_blocks(classes, n_j, n_i)

    with tile.TileContext(nc) as tc:
        with tc.tile_pool(name="const", bufs=1) as const:
            cos_t = const.tile([P, s], BF16)
            sin_t = const.tile([P, s], BF16)
            ones_r = const.tile([P, 1], BF16)
            nc.gpsimd.dma_start(ones_r[:], onesb[:])
            mblk_t = const.tile([P, nblk, SQ], BF16)
            qh_sb = const.tile([P, HPG, s], BF16)
            kh_sb = const.tile([P, HPG, s], BF16)
            vh_sb = const.tile([P, n_i, C], BF16)
            attn_t = const.tile([P, HPG, s], BF16)
            wo_t = const.tile([P, HPG, d], BF16)

            # ---------------- Phase A: QKV projections + RoPE ----------------
            with tc.tile_pool(name="wres", bufs=1) as wres, \
                 tc.tile_pool(name="xc", bufs=2) as xcp, \
                 tc.tile_pool(name="ptmp", bufs=4) as ptmp, \
                 tc.tile_pool(name="psW", bufs=HPG, space="PSUM") as psW, \
                 tc.tile_pool(name="psA", bufs=2, space="PSUM") as psA:
                wq_t = wres.tile([P, nkt, C], BF16)
                wk_t = wres.tile([P, nkt, C], BF16)
                wv_t = wres.tile([P, nkt, C], BF16)
                qk4 = nkt // 4
                qk8 = nkt // 8
                # critical-path loads, k-chunked: wq + x chunk 0 only.
                for q4 in range(4):
                    ksl = slice(q4 * qk4, (q4 + 1) * qk4)
                    nc.sync.dma_start(wq_t[:, ksl, :], wqT[:, ksl, :])
                xc_tiles = {}

                def get_xc(j):
                    if j not in xc_tiles:
                        xc_tiles[j] = xcp.tile([P, nkt, CH_A], BF16, tag="xc",
                                               name=f"xc{j}")
                    return xc_tiles[j]

                xc0 = get_xc(0)
                for q4 in range(4):
                    ksl = slice(q4 * qk4, (q4 + 1) * qk4)
                    nc.scalar.dma_start(xc0[:, ksl, :], xT[0][:, ksl, :])
                # gate wk/cos/sin on the last x0 chunk's arrival
                for q4 in range(4):
                    nc.vector.tensor_copy(wk_t[0:1, q4 * qk4, 0:1],
                                          xc0[0:1, nkt - 1, CH_A - 1:CH_A])
                nc.vector.tensor_copy(cos_t[0:1, 0:1],
                                      xc0[0:1, nkt - 1, CH_A - 1:CH_A])
                nc.vector.tensor_copy(sin_t[0:1, 0:1],
                                      xc0[0:1, nkt - 1, CH_A - 1:CH_A])
                for q4 in range(4):
                    ksl = slice(q4 * qk4, (q4 + 1) * qk4)
                    nc.gpsimd.dma_start(wk_t[:, ksl, :], wkT[:, ksl, :])
                nc.sync.dma_start(cos_t[:], cosP[:])
                nc.sync.dma_start(sin_t[:], sinSw[:])

                def rope(ps, dst, ct, sl, j, wt):
                    # RoPE (rotate-half layout):
                    #   out_top = x0*cos - x1*sin ; out_bot = x1*cos + x0*sin
                    # ACT stages PSUM->SBUF bf16 so DVE gets 2x mode
                    pc = ptmp.tile([P, CH_A], BF16, tag="pc")
                    nc.scalar.activation(pc[:], ps[:], COPY)
                    if j == 0 and wt is wq_t and ct == 0:
                        # release wv + x chunk 1 loads (needed ~25us in)
                        for q4 in range(4):
                            nc.vector.tensor_copy(
                                wv_t[0:1, q4 * qk4, 0:1], pc[0:1, 0:1])
                        for q4 in range(4):
                            ksl = slice(q4 * qk4, (q4 + 1) * qk4)
                            nc.gpsimd.dma_start(wv_t[:, ksl, :], wvT[:, ksl, :])
                    if j == 0 and wt is wq_t and ct == 1:
                        xc1 = get_xc(1)
                        nc.vector.tensor_copy(xc1[0:1, 0, 0:1], pc[0:1, 0:1])
                        nc.scalar.dma_start(xc1[:], xT[1])
                    t1 = ptmp.tile([P, CH_A], BF16, tag="t1")
                    nc.vector.tensor_tensor(t1[:], pc[:], cos_t[:, sl], MULT)
                    t2 = ptmp.tile([P, CH_A], BF16, tag="t2")
                    nc.vector.tensor_tensor(
                        t2[0:64, :], pc[64:128, :], sin_t[64:128, sl], MULT)
                    nc.vector.tensor_tensor(
                        t2[64:128, :], pc[0:64, :], sin_t[0:64, sl], MULT)
                    nc.vector.tensor_tensor(dst[:, ct, sl], t1[:], t2[:], ADD)

                for j in range(n_ja):
                    sl = slice(j * CH_A, (j + 1) * CH_A)
                    xc = get_xc(j)
                    if j > 1:
                        nc.scalar.dma_start(xc[:], xT[j])
                    if j == 0:
                        # Q projection in k-waves: wave kc needs only the
                        # kc-th wq/x k-chunk, so the PE starts on the first
                        # 0.5MB of each instead of waiting for the full 2MB.
                        pss = [psW.tile([P, CH_A], F32, tag="ps_w",
                                        name=f"psw{ct}") for ct in range(HPG)]
                        for kc in range(4):
                            for ct in range(HPG):
                                for k in range(kc * qk4, (kc + 1) * qk4):
                                    nc.tensor.matmul(
                                        pss[ct][:],
                                        wq_t[:, k, ct * P:(ct + 1) * P],
                                        xc[:, k, :],
                                        start=(k == 0), stop=(k == nkt - 1),
                                        skip_group_check=True)
                        for ct in range(HPG):
                            rope(pss[ct], qh_sb, ct, sl, j, wq_t)
                        qk_iter = ((wk_t, kh_sb),)
                    else:
                        qk_iter = ((wq_t, qh_sb), (wk_t, kh_sb))
                    for (wt, dst) in qk_iter:
                        for ct in range(HPG):
                            ps = psA.tile([P, CH_A], F32, tag="ps_qk")
                            for k in range(nkt):
                                nc.tensor.matmul(
                                    ps[:], wt[:, k, ct * P:(ct + 1) * P],
                                    xc[:, k, :],
                                    start=(k == 0), stop=(k == nkt - 1))
                            rope(ps, dst, ct, sl, j, wt)
                    for st2 in range(CH_A // P):
                        st = (j * CH_A) // P + st2
                        psv = psA.tile([P, C], F32, tag="ps_v")
                        for k in range(nkt):
                            nc.tensor.matmul(
                                psv[:], xc[:, k, st2 * P:(st2 + 1) * P],
                                wv_t[:, k, :],
                                start=(k == 0), stop=(k == nkt - 1))
                        nc.scalar.activation(vh_sb[:, st, :], psv[:], COPY)
                        if j == 0 and st2 == 0:
                            # release the late constant loads (wo/mblk)
                            nc.vector.tensor_copy(
                                wo_t[0:1, 0, 0:1], vh_sb[0:1, 0, 0:1])
                            nc.vector.tensor_copy(
                                mblk_t[0:1, 0, 0:1], vh_sb[0:1, 0, 0:1])
                            nc.gpsimd.dma_start(wo_t[:], woT[:])
                            nc.gpsimd.dma_start(mblk_t[:], mblk[:])

            # ---------------- Phase B: attention per head ----------------
            # flat pair-block list, software-pipelined with lookahead 1:
            # PE stream is ... sc(n+1), av(n), dn(n) ... so exp(n) on ACT
            # overlaps av(n-1)/dn(n-1)/sc(n+1) on the PE.
            flat = []
            for h in range(HPG):
                for (pidx, base_jq, wchunks, blocks, lo_last) in pairs:
                    nb = len(blocks)
                    for bi, blk in enumerate(blocks):
                        flat.append((h, pidx, base_jq, wchunks, blk,
                                     bi == 0, bi == nb - 1, bi == lo_last))

            with tc.tile_pool(name="pr", bufs=4) as prp, \
                 tc.tile_pool(name="sm", bufs=2) as smp, \
                 tc.tile_pool(name="psS", bufs=2, space="PSUM") as psS, \
                 tc.tile_pool(name="psB", bufs=1, space="PSUM") as psB:
                prs = {}
                gps = {}
                has_lo = set()
                dnpend = {}

                def pieces(cs, ce):
                    # matmul moving operand caps at 512 and PSUM writes must
                    # stay within a bank: split segments at the SQ boundary
                    out = []
                    while cs < ce:
                        pe_ = min(ce, (cs // SQ + 1) * SQ)
                        out.append((cs, pe_))
                        cs = pe_
                    return out

                def emit_score(idx):
                    h, pidx, bjq, wch, blk, first, last_, lo_l = flat[idx]
                    w = wch * SQ
                    sc = psS.tile([P, PW], F32, tag="sc")
                    pr = prp.tile([P, PW], BF16, tag="pr")
                    base = bjq * SQ
                    for (cs, ce, sflag) in blk['segs']:
                        for (ps_, pe_) in pieces(cs, ce):
                            nc.tensor.matmul(
                                sc[:, ps_:pe_],
                                kh_sb[:, h, blk['i'] * P:(blk['i'] + 1) * P],
                                qh_sb[:, h, base + ps_:base + pe_],
                                start=True, stop=True)
                        nc.scalar.activation(
                            pr[:, cs:ce], sc[:, cs:ce], EXP, scale=SCALE)
                    for (off, qoff, cls) in blk['pats']:
                        nc.vector.tensor_tensor(
                            pr[:, off + qoff:off + SQ],
                            pr[:, off + qoff:off + SQ],
                            mblk_t[:, cls, qoff:], MULT)
                    prs[idx] = pr

                def emit_accum(idx):
                    h, pidx, bjq, wch, blk, first, last_, lo_l = flat[idx]
                    w = wch * SQ
                    if first:
                        gps[(h, pidx)] = (
                            psB.tile([P, PW], F32, tag="at",
                                     name=f"at{h}_{pidx}"),
                            psB.tile([1, PW], F32, tag="dn",
                                     name=f"dn{h}_{pidx}"),
                            smp.tile([1, PW], F32, tag="dnsb",
                                     name=f"dnsb{h}_{pidx}"))
                    at_ps, dn_ps, dn_sb = gps[(h, pidx)]
                    pr = prs.pop(idx)
                    i = blk['i']
                    for (cs, ce, sflag) in blk['segs']:
                        for (ps_, pe_) in pieces(cs, ce):
                            nc.tensor.matmul(
                                at_ps[:, ps_:pe_],
                                vh_sb[:, i, h * HD:(h + 1) * HD],
                                pr[:, ps_:pe_],
                                start=sflag, stop=last_, skip_group_check=True)
                    role = blk.get('dnrole', 0)
                    if role == 1:
                        # defer: this block's probs are summed with the next
                        # block's (identical coverage) for one shared dn MM
                        dnpend[(h, pidx)] = (pr, blk['segs'])
                    else:
                        dn_src = pr
                        dn_segs = blk['segs']
                        if role == 2:
                            # start flags come from the earlier block (it may
                            # be the group's initializer)
                            pr0, dn_segs = dnpend.pop((h, pidx))
                            dn_src = prp.tile([P, PW], BF16, tag="prsum")
                            for (cs, ce, sflag) in blk['segs']:
                                nc.vector.tensor_tensor(
                                    dn_src[:, cs:ce], pr0[:, cs:ce],
                                    pr[:, cs:ce], ADD)
                        for (cs, ce, sflag) in dn_segs:
                            for (ps_, pe_) in pieces(cs, ce):
                                nc.tensor.matmul(
                                    dn_ps[:, ps_:pe_], ones_r[:],
                                    dn_src[:, ps_:pe_],
                                    start=sflag, stop=last_,
                                    skip_group_check=True)
                    if lo_l and not last_:
                        # the lo half is final: copy it out now, overlapped
                        # with the remaining hi-only blocks, so the PSUM
                        # buffers recycle fast at the group boundary
                        nc.vector.tensor_copy(
                            attn_t[:, h, bjq * SQ:(bjq + 1) * SQ],
                            at_ps[:, 0:SQ])
                        nc.scalar.activation(dn_sb[:, 0:SQ], dn_ps[:, 0:SQ],
                                             COPY)
                        has_lo.add((h, pidx))
                    if last_:
                        qsl = slice(bjq * SQ, bjq * SQ + w)
                        # hi-half (or full) copies, hoisted ahead of the next
                        # group's queue entries so the buffers free ASAP
                        hi0 = SQ if (h, pidx) in has_lo else 0
                        nc.vector.tensor_copy(
                            attn_t[:, h, bjq * SQ + hi0:bjq * SQ + w],
                            at_ps[:, hi0:w])
                        nc.scalar.activation(dn_sb[:, hi0:w],
                                             dn_ps[:, hi0:w], COPY)
                        # fold + invert denominators; gpsimd broadcast; DVE
                        # normalize (deferred -- not on the PE critical path)
                        nf = w // P
                        dn4 = smp.tile([PW // P, P], F32, tag="dn4")
                        nc.gpsimd.dma_start(dn4[0:nf, :], dn_sb[:, 0:w])
                        rc4 = smp.tile([PW // P, P], F32, tag="rc4")
                        nc.vector.reciprocal(rc4[0:nf, :], dn4[0:nf, :])
                        rc4b = smp.tile([PW // P, P], BF16, tag="rc4b")
                        nc.vector.tensor_copy(rc4b[0:nf, :], rc4[0:nf, :])
                        rc = smp.tile([1, PW], BF16, tag="rc")
                        nc.gpsimd.dma_start(rc[:, 0:w], rc4b[0:nf, :])
                        bc_sb = smp.tile([P, PW], BF16, tag="bcs")
                        nc.gpsimd.partition_broadcast(bc_sb[:, 0:w],
                                                      rc[:, 0:w])
                        nc.vector.tensor_tensor(
                            attn_t[:, h, qsl], attn_t[:, h, qsl],
                            bc_sb[:, 0:w], MULT)

                LA = 1
                nfl = len(flat)
                for idx in range(nfl):
                    emit_score(idx)
                    if idx - LA >= 0:
                        emit_accum(idx - LA)
                for idx in range(nfl - LA, nfl):
                    emit_accum(idx)

            # ---------------- Phase C: output projection ----------------
            with tc.tile_pool(name="og", bufs=2) as ogp, \
                 tc.tile_pool(name="psC", bufs=4, space="PSUM") as psC:
                for st in range(n_i):
                    og = ogp.tile([P, d], BF16, tag="og")
                    for dch in range(d // SQ):
                        po = psC.tile([P, SQ], F32, tag="po")
                        for ct in range(HPG):
                            nc.tensor.matmul(
                                po[:], attn_t[:, ct, st * P:(st + 1) * P],
                                wo_t[:, ct, dch * SQ:(dch + 1) * SQ],
                                start=(ct == 0), stop=(ct == HPG - 1))
                        nc.scalar.activation(
                            og[:, dch * SQ:(dch + 1) * SQ], po[:], COPY)
                        nc.sync.dma_start(
                            out[st * P:(st + 1) * P, dch * SQ:(dch + 1) * SQ],
                            og[:, dch * SQ:(dch + 1) * SQ])

    nc.compile()
    return nc


def _class_key(classes):
    def k(v):
        return v if isinstance(v, str) else tuple(v)
    return tuple(sorted((jk, k(v)) for jk, v in classes.items()))


def _prep_host(inputs):
    """Shard + transpose + bf16-convert the full inputs into 8 per-core maps."""
    x = np.asarray(inputs["x"], np.float32)
    wq = np.asarray(inputs["wq"], np.float32)
    wk = np.asarray(inputs["wk"], np.float32)
    wv = np.asarray(inputs["wv"], np.float32)
    wo = np.asarray(inputs["wo"], np.float32)
    cos = np.asarray(inputs["cos"], np.float32)
    sin = np.asarray(inputs["sin"], np.float32)
    mask = np.asarray(inputs["mask"], np.float32)
    start_p = int(inputs["start_p"])

    s = x.shape[1]
    cos_u = cos[start_p:start_p + s]          # (s, HD/2)
    sin_u = sin[start_p:start_p + s]

    # rotate-half channel permutation within each head: [evens, odds]
    perm = np.concatenate(
        [h * HD + np.concatenate([np.arange(0, HD, 2), np.arange(1, HD, 2)])
         for h in range(H)])

    cosP = np.ascontiguousarray(
        np.concatenate([cos_u.T, cos_u.T], axis=0)).astype(BF)     # (128, s)
    sinSw = np.ascontiguousarray(
        np.concatenate([sin_u.T, -sin_u.T], axis=0)).astype(BF)    # (128, s)

    classes, pats = _classify_mask(mask)
    onesb = np.ones((P, 1), BF)

    in_maps = []
    for b in range(B):
        xTp = _pre_x(np.ascontiguousarray(x[b].T))
        for g in range(GROUPS):
            rows = perm[g * C:(g + 1) * C]
            in_maps.append({
                "xT": xTp,
                "wqT": _pre_w(wq[rows, :].T),
                "wkT": _pre_w(wk[rows, :].T),
                "wvT": _pre_w(wv[g * C:(g + 1) * C, :].T),
                "woT": _pre_w(wo[:, g * C:(g + 1) * C].T),
                "cosP": cosP,
                "sinSw": sinSw,
                "mblk": np.ascontiguousarray(pats.transpose(1, 0, 2)),
                "onesb": onesb,
            })
    return in_maps, classes, pats


def _run(inputs, trace=False):
    in_maps, classes, pats = _prep_host(inputs)
    key = (pats.shape[0], _class_key(classes))
    if key not in _PROGRAM_CACHE:
        _PROGRAM_CACHE[key] = _build(classes, pats.shape[0])
    nc = _PROGRAM_CACHE[key]
    res = bass_utils.run_bass_kernel_spmd(
        nc, in_maps, core_ids=list(range(NCORES)), trace=trace)
    out = np.zeros((B, S, D), np.float32)
    for b in range(B):
        acc = res.results[b * GROUPS]["out"].astype(np.float32).copy()
        for g in range(1, GROUPS):
            acc += res.results[b * GROUPS + g]["out"].astype(np.float32)
        out[b] = acc
    return out, res


def kernel(**inputs):
    out, _ = _run(inputs, trace=False)
    return out


# revision 33
# speedup vs baseline: 1.0057x; 1.0004x over previous
"""Trainium2 Bass kernel for nn_Attention_51307679318359.

Multi-head attention (B=2, S=2048, D=2048, H=16, HD=128) with RoPE and an
additive mask, sharded over 8 NeuronCores as (batch x head-group): each core
computes 1 batch and 4 heads (512 channels), producing a partial output that
the host sums over head-groups.

v4 (bf16, SBUF-resident, pair-wide phase B):
- all operands bf16 (host-converted); Q/K/V and attn stay SBUF-resident.
- phase B processes sq in 1024-wide jq-PAIRS: one wide scores matmul, one
  wide exp, one wide AV and one wide denominator matmul per (pair, sk-tile)
  block -- half the instruction count of 512-wide blocks.  Software
  pipelined (lookahead 1 pair-block) so the PE never waits on ACT.
- softmax normalization via gpsimd partition_broadcast (no PE broadcast
  matmul); causal diagonal blocks are column-trimmed.
- phase A runs the first Q projection in k-waves so the PE starts as soon
  as the first 0.5MB of wq/x lands; all non-critical DMAs are gated behind
  early compute via tiny WAW corner-writes so they don't steal HBM
  bandwidth from the critical-path loads.
"""

import math

import numpy as np
import ml_dtypes

import concourse.bass as bass
import concourse.mybir as mybir
import concourse.tile as tile
from concourse import bacc
from concourse import bass_utils

F32 = mybir.dt.float32
F32R = mybir.dt.float32r
BF16 = mybir.dt.bfloat16
ADD = mybir.AluOpType.add
MULT = mybir.AluOpType.mult
COPY = mybir.ActivationFunctionType.Copy
EXP = mybir.ActivationFunctionType.Exp

B, S, D = 2, 2048, 2048
H, HD = 16, 128
NCORES = 8
GROUPS = NCORES // B          # 4 head-groups
HPG = H // GROUPS             # 4 heads per group
C = HPG * HD                  # 512 per-core channels
P = 128
CH_A = 512                    # phase-A s-chunk width
SQ = 512                      # mask-classification sq-chunk width
PW = 1024                     # phase-B pair window (2 * SQ)
SCALE = 1.0 / math.sqrt(HD)
NEG_THRESH = -1e8             # "masked out" threshold
BF = ml_dtypes.bfloat16

_PROGRAM_CACHE = {}


def _pre_w(wT):
    """(d, c) row-major -> (128, d//128, c) partition-major contiguous bf16."""
    d, c = wT.shape
    return np.ascontiguousarray(
        wT.reshape(d // P, P, c).transpose(1, 0, 2)).astype(BF)


def _pre_x(xT):
    """(d, s) -> (s//CH_A, 128, d//128, CH_A) chunk-major contiguous bf16."""
    d, s = xT.shape
    return np.ascontiguousarray(
        xT.reshape(d // P, P, s // CH_A, CH_A).transpose(2, 1, 0, 3)).astype(BF)


def _classify_mask(mask):
    """Classify transposed-mask blocks (sk-tile i x sq-chunk j).  Returns
    (classes, patterns): classes[(j, i)] is 'skip' | ('plain', qoff) |
    (pat_idx, qoff) where qoff counts leading fully-masked sq columns of the
    block (trimmed from all device matmuls).  patterns is (nblk, 128, SQ)
    bf16 holding exp(maskT block)."""
    maskT = np.ascontiguousarray(mask.T)
    n_j = mask.shape[0] // SQ
    n_i = mask.shape[0] // P
    classes = {}
    patterns = []
    pat_idx = {}

    def add_pattern(blk):
        key = blk.tobytes()
        if key not in pat_idx:
            pat_idx[key] = len(patterns)
            with np.errstate(over='ignore'):
                patterns.append(np.exp(blk.astype(np.float64)).astype(BF))
        return pat_idx[key]

    for j in range(n_j):
        for i in range(n_i):
            blk = maskT[i * P:(i + 1) * P, j * SQ:(j + 1) * SQ]
            dead_col = np.all(blk <= NEG_THRESH, axis=0)  # (SQ,)
            if np.all(dead_col):
                classes[(j, i)] = 'skip'
                continue
            qoff = 0
            while dead_col[qoff]:
                qoff += 1
            rest = blk[:, qoff:]
            if np.all(rest == 0.0):
                classes[(j, i)] = ('plain', qoff)
            else:
                classes[(j, i)] = (add_pattern(blk), qoff)
    # every sq position must keep at least one live sk tile, else softmax
    # denominators vanish; fall back to no skipping in that degenerate case
    if any(all(classes[(j, i)] == 'skip' for i in range(n_i)) for j in range(n_j)):
        for j in range(n_j):
            for i in range(n_i):
                if classes[(j, i)] == 'skip':
                    blk = maskT[i * P:(i + 1) * P, j * SQ:(j + 1) * SQ]
                    classes[(j, i)] = (add_pattern(blk), 0)
    # the first live block of each sq chunk must cover its full width (it
    # initializes the PSUM accumulation); widen it to qoff=0, which requires
    # the pattern (leading columns are fully masked -> exp(mask)=0 there)
    for j in range(n_j):
        for i in range(n_i):
            cls = classes[(j, i)]
            if cls == 'skip':
                continue
            if cls[1] != 0:
                blk = maskT[i * P:(i + 1) * P, j * SQ:(j + 1) * SQ]
                classes[(j, i)] = (add_pattern(blk), 0)
            break
    pats = np.stack(patterns, 0).astype(BF) if patterns else \
        np.zeros((1, P, SQ), BF)
    return classes, pats


def _pair_blocks(classes, n_j, n_i):
    """Group sq chunks into pairs and build per-(pair, sk-tile) blocks.

    Returns pairs: list of (pair_idx, base_jq, width_chunks, blocks) where
    blocks is a list of dicts with:
      i, segs: [(cs, ce, start_flag)], pats: [(off, qoff, cls)]
    cs/ce are columns relative to the pair window.  A single contiguous
    segment is merged across both halves when possible."""
    pairs = []
    pj = 0
    pidx = 0
    while pj < n_j:
        wchunks = 2 if pj + 1 < n_j else 1
        jqs = list(range(pj, pj + wchunks))
        half_first = {jq: True for jq in jqs}
        blocks = []
        for i in range(n_i):
            halves = []
            for hh, jq in enumerate(jqs):
                cls = classes[(jq, i)]
                if cls == 'skip':
                    continue
                pat, qoff = cls
                halves.append((hh, pat, qoff))
            if not halves:
                continue
            segs = []
            pats_ = []
            firsts = [half_first[jqs[h[0]]] for h in halves]
            merged = (len(halves) == 2 and halves[1][2] == 0
                      and firsts[0] == firsts[1])
            if merged:
                hh0, pat0, qoff0 = halves[0]
                segs.append((qoff0, 2 * SQ, firsts[0]))
            else:
                for hh, pat, qoff in halves:
                    segs.append((hh * SQ + qoff, (hh + 1) * SQ,
                                 half_first[jqs[hh]]))
            for hh, pat, qoff in halves:
                if isinstance(pat, int):
                    pats_.append((hh * SQ, qoff, pat))
                half_first[jqs[hh]] = False
            blocks.append(dict(i=i, segs=segs, pats=pats_))
        # last block that writes any column < SQ (the lo half): its copies
        # can be emitted early, overlapping the remaining hi-only blocks
        lo_last = -1
        if wchunks == 2:
            for bi, blk in enumerate(blocks):
                if any(cs < SQ for (cs, ce, sf) in blk['segs']):
                    lo_last = bi
            if lo_last == len(blocks) - 1:
                lo_last = -1
        # pair up adjacent blocks with identical segment coverage: their
        # probs are summed on the DVE and contribute a single denominator
        # matmul (dnrole: 0 = solo, 1 = defer into next, 2 = paired-with-prev)
        bi = 0
        while bi < len(blocks):
            b0 = blocks[bi]
            if (bi + 1 < len(blocks)
                    and blocks[bi + 1]['segs'] == b0['segs']
                    and all(not sf for (_, _, sf) in blocks[bi + 1]['segs'])):
                b0['dnrole'] = 1
                blocks[bi + 1]['dnrole'] = 2
                bi += 2
            else:
                b0['dnrole'] = 0
                bi += 1
        pairs.append((pidx, pj, wchunks, blocks, lo_last))
        pj += wchunks
        pidx += 1
    return pairs


def _build(classes, nblk, s=S, d=D):
    """Build + compile the per-core SPMD program."""
    nkt = d // P
    n_j = s // SQ
    n_i = s // P
    n_ja = s // CH_A

    nc = bacc.Bacc("TRN2", target_bir_lowering=False, debug=False)
    xT = nc.dram_tensor("xT", (n_ja, P, nkt, CH_A), BF16, kind="ExternalInput")
    wqT = nc.dram_tensor("wqT", (P, nkt, C), BF16, kind="ExternalInput")
    wkT = nc.dram_tensor("wkT", (P, nkt, C), BF16, kind="ExternalInput")
    wvT = nc.dram_tensor("wvT", (P, nkt, C), BF16, kind="ExternalInput")
    woT = nc.dram_tensor("woT", (P, HPG, d), BF16, kind="ExternalInput")
    cosP = nc.dram_tensor("cosP", (HD, s), BF16, kind="ExternalInput")
    sinSw = nc.dram_tensor("sinSw", (HD, s), BF16, kind="ExternalInput")
    mblk = nc.dram_tensor("mblk", (P, nblk, SQ), BF16, kind="ExternalInput")
    onesb = nc.dram_tensor("onesb", (P, 1), BF16, kind="ExternalInput")
    out = nc.dram_tensor("out", (s, d), BF16, kind="ExternalOutput")

    pairs = _pair_blocks(classes, n_j, n_i)

    with tile.TileContext(nc) as tc:
        with tc.tile_pool(name="const", bufs=1) as const:
            cos_t = const.tile([P, s], BF16)
            sin_t = const.tile([P, s], BF16)
            ones_r = const.tile([P, 1], BF16)
            nc.gpsimd.dma_start(ones_r[:], onesb[:])
            mblk_t = const.tile([P, nblk, SQ], BF16)
            qh_sb = const.tile([P, HPG, s], BF16)
            kh_sb = const.tile([P, HPG, s], BF16)
            vh_sb = const.tile([P, n_i, C], BF16)
            attn_t = const.tile([P, HPG, s], BF16)
            wo_t = const.tile([P, HPG, d], BF16)

            # ---------------- Phase A: QKV projections + RoPE ----------------
            with tc.tile_pool(name="wres", bufs=1) as wres, \
                 tc.tile_pool(name="xc", bufs=2) as xcp, \
                 tc.tile_pool(name="ptmp", bufs=4) as ptmp, \
                 tc.tile_pool(name="psW", bufs=HPG, space="PSUM") as psW, \
                 tc.tile_pool(name="psA", bufs=2, space="PSUM") as psA:
                wq_t = wres.tile([P, nkt, C], BF16)
                wk_t = wres.tile([P, nkt, C], BF16)
                wv_t = wres.tile([P, nkt, C], BF16)
                qk4 = nkt // 4
                qk8 = nkt // 8
                # critical-path loads, k-chunked: wq + x chunk 0 only.
                for q4 in range(4):
                    ksl = slice(q4 * qk4, (q4 + 1) * qk4)
                    nc.sync.dma_start(wq_t[:, ksl, :], wqT[:, ksl, :])
                xc_tiles = {}

                def get_xc(j):
                    if j not in xc_tiles:
                        xc_tiles[j] = xcp.tile([P, nkt, CH_A], BF16, tag="xc",
                                               name=f"xc{j}")
                    return xc_tiles[j]

                xc0 = get_xc(0)
                for q4 in range(4):
                    ksl = slice(q4 * qk4, (q4 + 1) * qk4)
                    nc.scalar.dma_start(xc0[:, ksl, :], xT[0][:, ksl, :])
                # gate wk/cos/sin on the last x0 chunk's arrival
                for q4 in range(4):
                    nc.vector.tensor_copy(wk_t[0:1, q4 * qk4, 0:1],
                                          xc0[0:1, nkt - 1, CH_A - 1:CH_A])
                nc.vector.tensor_copy(cos_t[0:1, 0:1],
                                      xc0[0:1, nkt - 1, CH_A - 1:CH_A])
                nc.vector.tensor_copy(sin_t[0:1, 0:1],
                                      xc0[0:1, nkt - 1, CH_A - 1:CH_A])
                for q4 in range(4):
                    ksl = slice(q4 * qk4, (q4 + 1) * qk4)
                    nc.gpsimd.dma_start(wk_t[:, ksl, :], wkT[:, ksl, :])
                nc.sync.dma_start(cos_t[:], cosP[:])
                nc.sync.dma_start(sin_t[:], sinSw[:])

                def rope(ps, dst, ct, sl, j, wt):
                    # RoPE (rotate-half layout):
                    #   out_top = x0*cos - x1*sin ; out_bot = x1*cos + x0*sin
                    # ACT stages PSUM->SBUF bf16 so DVE gets 2x mode
                    pc = ptmp.tile([P, CH_A], BF16, tag="pc")
                    nc.scalar.activation(pc[:], ps[:], COPY)
                    if j == 0 and wt is wq_t and ct == 0:
                        # release wv + x chunk 1 loads (needed ~25us in)
                        for q4 in range(4):
                            nc.vector.tensor_copy(
                                wv_t[0:1, q4 * qk4, 0:1], pc[0:1, 0:1])
                        for q4 in range(4):
                            ksl = slice(q4 * qk4, (q4 + 1) * qk4)
                            nc.gpsimd.dma_start(wv_t[:, ksl, :], wvT[:, ksl, :])
                    if j == 0 and wt is wq_t and ct == 1:
                        xc1 = get_xc(1)
                        nc.vector.tensor_copy(xc1[0:1, 0, 0:1], pc[0:1, 0:1])
                        nc.scalar.dma_start(xc1[:], xT[1])
                    t1 = ptmp.tile([P, CH_A], BF16, tag="t1")
                    nc.vector.tensor_tensor(t1[:], pc[:], cos_t[:, sl], MULT)
                    t2 = ptmp.tile([P, CH_A], BF16, tag="t2")
                    nc.vector.tensor_tensor(
                        t2[0:64, :], pc[64:128, :], sin_t[64:128, sl], MULT)
                    nc.vector.tensor_tensor(
                        t2[64:128, :], pc[0:64, :], sin_t[0:64, sl], MULT)
                    nc.vector.tensor_tensor(dst[:, ct, sl], t1[:], t2[:], ADD)

                for j in range(n_ja):
                    sl = slice(j * CH_A, (j + 1) * CH_A)
                    xc = get_xc(j)
                    if j > 1:
                        nc.scalar.dma_start(xc[:], xT[j])
                    if j == 0:
                        # Q projection in k-waves: wave kc needs only the
                        # kc-th wq/x k-chunk, so the PE starts on the first
                        # 0.5MB of each instead of waiting for the full 2MB.
                        pss = [psW.tile([P, CH_A], F32, tag="ps_w",
                                        name=f"psw{ct}") for ct in range(HPG)]
                        for kc in range(4):
                            for ct in range(HPG):
                                for k in range(kc * qk4, (kc + 1) * qk4):
                                    nc.tensor.matmul(
                                        pss[ct][:],
                                        wq_t[:, k, ct * P:(ct + 1) * P],
                                        xc[:, k, :],
                                        start=(k == 0), stop=(k == nkt - 1),
                                        skip_group_check=True)
                        for ct in range(HPG):
                            rope(pss[ct], qh_sb, ct, sl, j, wq_t)
                        qk_iter = ((wk_t, kh_sb),)
                    else:
                        qk_iter = ((wq_t, qh_sb), (wk_t, kh_sb))
                    for (wt, dst) in qk_iter:
                        for ct in range(HPG):
                            ps = psA.tile([P, CH_A], F32, tag="ps_qk")
                            for k in range(nkt):
                                nc.tensor.matmul(
                                    ps[:], wt[:, k, ct * P:(ct + 1) * P],
                                    xc[:, k, :],
                                    start=(k == 0), stop=(k == nkt - 1))
                            rope(ps, dst, ct, sl, j, wt)
                    for st2 in range(CH_A // P):
                        st = (j * CH_A) // P + st2
                        psv = psA.tile([P, C], F32, tag="ps_v")
                        for k in range(nkt):
                            nc.tensor.matmul(
                                psv[:], xc[:, k, st2 * P:(st2 + 1) * P],
                                wv_t[:, k, :],
                                start=(k == 0), stop=(k == nkt - 1))
                        nc.scalar.activation(vh_sb[:, st, :], psv[:], COPY)
                        if j == 0 and st2 == 0:
                            # release the late constant loads (wo/mblk)
                            nc.vector.tensor_copy(
                                wo_t[0:1, 0, 0:1], vh_sb[0:1, 0, 0:1])
                            nc.vector.tensor_copy(
                                mblk_t[0:1, 0, 0:1], vh_sb[0:1, 0, 0:1])
                            nc.gpsimd.dma_start(wo_t[:], woT[:])
                            nc.gpsimd.dma_start(mblk_t[:], mblk[:])

            # ---------------- Phase B: attention per head ----------------
            # flat pair-block list, software-pipelined with lookahead 1:
            # PE stream is ... sc(n+1), av(n), dn(n) ... so exp(n) on ACT
            # overlaps av(n-1)/dn(n-1)/sc(n+1) on the PE.
            flat = []
            for h in range(HPG):
                for (pidx, base_jq, wchunks, blocks, lo_last) in pairs:
                    nb = len(blocks)
                    for bi, blk in enumerate(blocks):
                        flat.append((h, pidx, base_jq, wchunks, blk,
                                     bi == 0, bi == nb - 1, bi == lo_last))

            with tc.tile_pool(name="pr", bufs=4) as prp, \
                 tc.tile_pool(name="sm", bufs=2) as smp, \
                 tc.tile_pool(name="psS", bufs=2, space="PSUM") as psS, \
                 tc.tile_pool(name="psB", bufs=1, space="PSUM") as psB:
                prs = {}
                gps = {}
                has_lo = set()
                dnpend = {}

                def pieces(cs, ce):
                    # matmul moving operand caps at 512 and PSUM writes must
                    # stay within a bank: split segments at the SQ boundary
                    out = []
                    while cs < ce:
                        pe_ = min(ce, (cs // SQ + 1) * SQ)
                        out.append((cs, pe_))
                        cs = pe_
                    return out

                def emit_score(idx):
                    h, pidx, bjq, wch, blk, first, last_, lo_l = flat[idx]
                    w = wch * SQ
                    sc = psS.tile([P, PW], F32, tag="sc")
                    pr = prp.tile([P, PW], BF16, tag="pr")
                    base = bjq * SQ
                    for (cs, ce, sflag) in blk['segs']:
                        for (ps_, pe_) in pieces(cs, ce):
                            nc.tensor.matmul(
                                sc[:, ps_:pe_],
                                kh_sb[:, h, blk['i'] * P:(blk['i'] + 1) * P],
                                qh_sb[:, h, base + ps_:base + pe_],
                                start=True, stop=True)
                        nc.scalar.activation(
                            pr[:, cs:ce], sc[:, cs:ce], EXP, scale=SCALE)
                    for (off, qoff, cls) in blk['pats']:
                        nc.vector.tensor_tensor(
                            pr[:, off + qoff:off + SQ],
                            pr[:, off + qoff:off + SQ],
                            mblk_t[:, cls, qoff:], MULT)
                    prs[idx] = pr

                def emit_accum(idx):
                    h, pidx, bjq, wch, blk, first, last_, lo_l = flat[idx]
                    w = wch * SQ
                    if first:
                        gps[(h, pidx)] = (
                            psB.tile([P, PW], F32, tag="at",
                                     name=f"at{h}_{pidx}"),
                            psB.tile([1, PW], F32, tag="dn",
                                     name=f"dn{h}_{pidx}"),
                            smp.tile([1, PW], F32, tag="dnsb",
                                     name=f"dnsb{h}_{pidx}"))
                    at_ps, dn_ps, dn_sb = gps[(h, pidx)]
                    pr = prs.pop(idx)
                    i = blk['i']
                    for (cs, ce, sflag) in blk['segs']:
                        for (ps_, pe_) in pieces(cs, ce):
                            nc.tensor.matmul(
                                at_ps[:, ps_:pe_],
                                vh_sb[:, i, h * HD:(h + 1) * HD],
                                pr[:, ps_:pe_],
                                start=sflag, stop=last_, skip_group_check=True)
                    role = blk.get('dnrole', 0)
                    if role == 1:
                        # defer: this block's probs are summed with the next
                        # block's (identical coverage) for one shared dn MM
                        dnpend[(h, pidx)] = (pr, blk['segs'])
                    else:
                        dn_src = pr
                        dn_segs = blk['segs']
                        if role == 2:
                            # start flags come from the earlier block (it may
                            # be the group's initializer)
                            pr0, dn_segs = dnpend.pop((h, pidx))
                            dn_src = prp.tile([P, PW], BF16, tag="prsum")
                            for (cs, ce, sflag) in blk['segs']:
                                nc.vector.tensor_tensor(
                                    dn_src[:, cs:ce], pr0[:, cs:ce],
                                    pr[:, cs:ce], ADD)
                        for (cs, ce, sflag) in dn_segs:
                            for (ps_, pe_) in pieces(cs, ce):
                                nc.tensor.matmul(
                                    dn_ps[:, ps_:pe_], ones_r[:],
                                    dn_src[:, ps_:pe_],
                                    start=sflag, stop=last_,
                                    skip_group_check=True)
                    if lo_l and not last_:
                        # the lo half is final: copy it out now, overlapped
                        # with the remaining hi-only blocks, so the PSUM
                        # buffers recycle fast at the group boundary
                        nc.vector.tensor_copy(
                            attn_t[:, h, bjq * SQ:(bjq + 1) * SQ],
                            at_ps[:, 0:SQ])
                        nc.scalar.activation(dn_sb[:, 0:SQ], dn_ps[:, 0:SQ],
                                             COPY)
                        has_lo.add((h, pidx))
                    if last_:
                        qsl = slice(bjq * SQ, bjq * SQ + w)
                        # hi-half (or full) copies, hoisted ahead of the next
                        # group's queue entries so the buffers free ASAP
                        hi0 = SQ if (h, pidx) in has_lo else 0
                        nc.vector.tensor_copy(
                            attn_t[:, h, bjq * SQ + hi0:bjq * SQ + w],
                            at_ps[:, hi0:w])
                        nc.scalar.activation(dn_sb[:, hi0:w],
                                             dn_ps[:, hi0:w], COPY)
                        # fold + invert denominators; gpsimd broadcast; DVE
                        # normalize (deferred -- not on the PE critical path)
                        nf = w // P
                        dn4 = smp.tile([PW // P, P], F32, tag="dn4")
                        nc.gpsimd.dma_start(dn4[0:nf, :], dn_sb[:, 0:w])
                        rc4 = smp.tile([PW // P, P], F32, tag="rc4")
                        nc.vector.reciprocal(rc4[0:nf, :], dn4[0:nf, :])
                        rc4b = smp.tile([PW // P, P], BF16, tag="rc4b")
                        nc.vector.tensor_copy(rc4b[0:nf, :], rc4[0:nf, :])
                        rc = smp.tile([1, PW], BF16, tag="rc")
                        nc.gpsimd.dma_start(rc[:, 0:w], rc4b[0:nf, :])
                        bc_sb = smp.tile([P, PW], BF16, tag="bcs")
                        nc.gpsimd.partition_broadcast(bc_sb[:, 0:w],
                                                      rc[:, 0:w])
                        nc.vector.tensor_tensor(
                            attn_t[:, h, qsl], attn_t[:, h, qsl],
                            bc_sb[:, 0:w], MULT)

                LA = 1
                nfl = len(flat)
                for idx in range(nfl):
                    emit_score(idx)
                    if idx - LA >= 0:
                        emit_accum(idx - LA)
                for idx in range(nfl - LA, nfl):
                    emit_accum(idx)

                # ---------------- Phase C: output projection ----------------
                # emitted inside the phase-B pool scope: po reuses the sc
                # PSUM tiles, avoiding a pool-close barrier between phases
                with tc.tile_pool(name="og", bufs=2) as ogp:
                    for st in range(n_i):
                        og = ogp.tile([P, d], BF16, tag="og")
                        for dch in range(d // SQ):
                            po = psS.tile([P, PW], F32, tag="sc",
                                          name=f"po{st}_{dch}")
                            for ct in range(HPG):
                                nc.tensor.matmul(
                                    po[:, 0:SQ],
                                    attn_t[:, ct, st * P:(st + 1) * P],
                                    wo_t[:, ct, dch * SQ:(dch + 1) * SQ],
                                    start=(ct == 0), stop=(ct == HPG - 1))
                            nc.scalar.activation(
                                og[:, dch * SQ:(dch + 1) * SQ], po[:, 0:SQ],
                                COPY)
                            nc.sync.dma_start(
                                out[st * P:(st + 1) * P,
                                    dch * SQ:(dch + 1) * SQ],
                                og[:, dch * SQ:(dch + 1) * SQ])

    nc.compile()
    return nc


def _class_key(classes):
    def k(v):
        return v if isinstance(v, str) else tuple(v)
    return tuple(sorted((jk, k(v)) for jk, v in classes.items()))


def _prep_host(inputs):
    """Shard + transpose + bf16-convert the full inputs into 8 per-core maps."""
    x = np.asarray(inputs["x"], np.float32)
    wq = np.asarray(inputs["wq"], np.float32)
    wk = np.asarray(inputs["wk"], np.float32)
    wv = np.asarray(inputs["wv"], np.float32)
    wo = np.asarray(inputs["wo"], np.float32)
    cos = np.asarray(inputs["cos"], np.float32)
    sin = np.asarray(inputs["sin"], np.float32)
    mask = np.asarray(inputs["mask"], np.float32)
    start_p = int(inputs["start_p"])

    s = x.shape[1]
    cos_u = cos[start_p:start_p + s]          # (s, HD/2)
    sin_u = sin[start_p:start_p + s]

    # rotate-half channel permutation within each head: [evens, odds]
    perm = np.concatenate(
        [h * HD + np.concatenate([np.arange(0, HD, 2), np.arange(1, HD, 2)])
         for h in range(H)])

    cosP = np.ascontiguousarray(
        np.concatenate([cos_u.T, cos_u.T], axis=0)).astype(BF)     # (128, s)
    sinSw = np.ascontiguousarray(
        np.concatenate([sin_u.T, -sin_u.T], axis=0)).astype(BF)    # (128, s)

    classes, pats = _classify_mask(mask)
    onesb = np.ones((P, 1), BF)

    in_maps = []
    for b in range(B):
        xTp = _pre_x(np.ascontiguousarray(x[b].T))
        for g in range(GROUPS):
            rows = perm[g * C:(g + 1) * C]
            in_maps.append({
                "xT": xTp,
                "wqT": _pre_w(wq[rows, :].T),
                "wkT": _pre_w(wk[rows, :].T),
                "wvT": _pre_w(wv[g * C:(g + 1) * C, :].T),
                "woT": _pre_w(wo[:, g * C:(g + 1) * C].T),
                "cosP": cosP,
                "sinSw": sinSw,
                "mblk": np.ascontiguousarray(pats.transpose(1, 0, 2)),
                "onesb": onesb,
            })
    return in_maps, classes, pats


def _run(inputs, trace=False):
    in_maps, classes, pats = _prep_host(inputs)
    key = (pats.shape[0], _class_key(classes))
    if key not in _PROGRAM_CACHE:
        _PROGRAM_CACHE[key] = _build(classes, pats.shape[0])
    nc = _PROGRAM_CACHE[key]
    res = bass_utils.run_bass_kernel_spmd(
        nc, in_maps, core_ids=list(range(NCORES)), trace=trace)
    out = np.zeros((B, S, D), np.float32)
    for b in range(B):
        acc = res.results[b * GROUPS]["out"].astype(np.float32).copy()
        for g in range(1, GROUPS):
            acc += res.results[b * GROUPS + g]["out"].astype(np.float32)
        out[b] = acc
    return out, res


def kernel(**inputs):
    out, _ = _run(inputs, trace=False)
    return out


# revision 34
# speedup vs baseline: 1.0168x; 1.0110x over previous
"""Trainium2 Bass kernel for nn_Attention_51307679318359.

Multi-head attention (B=2, S=2048, D=2048, H=16, HD=128) with RoPE and an
additive mask, sharded over 8 NeuronCores as (batch x head-group): each core
computes 1 batch and 4 heads (512 channels), producing a partial output that
the host sums over head-groups.

v4 (bf16, SBUF-resident, pair-wide phase B):
- all operands bf16 (host-converted); Q/K/V and attn stay SBUF-resident.
- phase B processes sq in 1024-wide jq-PAIRS: one wide scores matmul, one
  wide exp, one wide AV and one wide denominator matmul per (pair, sk-tile)
  block -- half the instruction count of 512-wide blocks.  Software
  pipelined (lookahead 1 pair-block) so the PE never waits on ACT.
- softmax normalization via gpsimd partition_broadcast (no PE broadcast
  matmul); causal diagonal blocks are column-trimmed.
- phase A runs the first Q projection in k-waves so the PE starts as soon
  as the first 0.5MB of wq/x lands; all non-critical DMAs are gated behind
  early compute via tiny WAW corner-writes so they don't steal HBM
  bandwidth from the critical-path loads.
"""

import math

import numpy as np
import ml_dtypes

import concourse.bass as bass
import concourse.mybir as mybir
import concourse.tile as tile
from concourse import bacc
from concourse import bass_utils

F32 = mybir.dt.float32
F32R = mybir.dt.float32r
BF16 = mybir.dt.bfloat16
ADD = mybir.AluOpType.add
MULT = mybir.AluOpType.mult
COPY = mybir.ActivationFunctionType.Copy
EXP = mybir.ActivationFunctionType.Exp

B, S, D = 2, 2048, 2048
H, HD = 16, 128
NCORES = 8
GROUPS = NCORES // B          # 4 head-groups
HPG = H // GROUPS             # 4 heads per group
C = HPG * HD                  # 512 per-core channels
P = 128
CH_A = 512                    # phase-A s-chunk width
SQ = 512                      # mask-classification sq-chunk width
PW = 1024                     # phase-B pair window (2 * SQ)
SCALE = 1.0 / math.sqrt(HD)
NEG_THRESH = -1e8             # "masked out" threshold
BF = ml_dtypes.bfloat16

_PROGRAM_CACHE = {}


def _pre_w(wT):
    """(d, c) row-major -> (128, d//128, c) partition-major contiguous bf16."""
    d, c = wT.shape
    return np.ascontiguousarray(
        wT.reshape(d // P, P, c).transpose(1, 0, 2)).astype(BF)


def _pre_x(xT):
    """(d, s) -> (s//CH_A, 128, d//128, CH_A) chunk-major contiguous bf16."""
    d, s = xT.shape
    return np.ascontiguousarray(
        xT.reshape(d // P, P, s // CH_A, CH_A).transpose(2, 1, 0, 3)).astype(BF)


def _classify_mask(mask):
    """Classify transposed-mask blocks (sk-tile i x sq-chunk j).  Returns
    (classes, patterns): classes[(j, i)] is 'skip' | ('plain', qoff) |
    (pat_idx, qoff) where qoff counts leading fully-masked sq columns of the
    block (trimmed from all device matmuls).  patterns is (nblk, 128, SQ)
    bf16 holding exp(maskT block)."""
    maskT = np.ascontiguousarray(mask.T)
    n_j = mask.shape[0] // SQ
    n_i = mask.shape[0] // P
    classes = {}
    patterns = []
    pat_idx = {}

    def add_pattern(blk):
        key = blk.tobytes()
        if key not in pat_idx:
            pat_idx[key] = len(patterns)
            with np.errstate(over='ignore'):
                patterns.append(np.exp(blk.astype(np.float64)).astype(BF))
        return pat_idx[key]

    for j in range(n_j):
        for i in range(n_i):
            blk = maskT[i * P:(i + 1) * P, j * SQ:(j + 1) * SQ]
            dead_col = np.all(blk <= NEG_THRESH, axis=0)  # (SQ,)
            if np.all(dead_col):
                classes[(j, i)] = 'skip'
                continue
            qoff = 0
            while dead_col[qoff]:
                qoff += 1
            rest = blk[:, qoff:]
            if np.all(rest == 0.0):
                classes[(j, i)] = ('plain', qoff)
            else:
                classes[(j, i)] = (add_pattern(blk), qoff)
    # every sq position must keep at least one live sk tile, else softmax
    # denominators vanish; fall back to no skipping in that degenerate case
    if any(all(classes[(j, i)] == 'skip' for i in range(n_i)) for j in range(n_j)):
        for j in range(n_j):
            for i in range(n_i):
                if classes[(j, i)] == 'skip':
                    blk = maskT[i * P:(i + 1) * P, j * SQ:(j + 1) * SQ]
                    classes[(j, i)] = (add_pattern(blk), 0)
    # the first live block of each sq chunk must cover its full width (it
    # initializes the PSUM accumulation); widen it to qoff=0, which requires
    # the pattern (leading columns are fully masked -> exp(mask)=0 there)
    for j in range(n_j):
        for i in range(n_i):
            cls = classes[(j, i)]
            if cls == 'skip':
                continue
            if cls[1] != 0:
                blk = maskT[i * P:(i + 1) * P, j * SQ:(j + 1) * SQ]
                classes[(j, i)] = (add_pattern(blk), 0)
            break
    pats = np.stack(patterns, 0).astype(BF) if patterns else \
        np.zeros((1, P, SQ), BF)
    return classes, pats


def _pair_blocks(classes, n_j, n_i):
    """Group sq chunks into pairs and build per-(pair, sk-tile) blocks.

    Returns pairs: list of (pair_idx, base_jq, width_chunks, blocks) where
    blocks is a list of dicts with:
      i, segs: [(cs, ce, start_flag)], pats: [(off, qoff, cls)]
    cs/ce are columns relative to the pair window.  A single contiguous
    segment is merged across both halves when possible."""
    pairs = []
    pj = 0
    pidx = 0
    while pj < n_j:
        wchunks = 2 if pj + 1 < n_j else 1
        jqs = list(range(pj, pj + wchunks))
        half_first = {jq: True for jq in jqs}
        blocks = []
        for i in range(n_i):
            halves = []
            for hh, jq in enumerate(jqs):
                cls = classes[(jq, i)]
                if cls == 'skip':
                    continue
                pat, qoff = cls
                halves.append((hh, pat, qoff))
            if not halves:
                continue
            segs = []
            pats_ = []
            firsts = [half_first[jqs[h[0]]] for h in halves]
            merged = (len(halves) == 2 and halves[1][2] == 0
                      and firsts[0] == firsts[1])
            if merged:
                hh0, pat0, qoff0 = halves[0]
                segs.append((qoff0, 2 * SQ, firsts[0]))
            else:
                for hh, pat, qoff in halves:
                    segs.append((hh * SQ + qoff, (hh + 1) * SQ,
                                 half_first[jqs[hh]]))
            for hh, pat, qoff in halves:
                if isinstance(pat, int):
                    pats_.append((hh * SQ, qoff, pat))
                half_first[jqs[hh]] = False
            blocks.append(dict(i=i, segs=segs, pats=pats_))
        # last block that writes any column < SQ (the lo half): its copies
        # can be emitted early, overlapping the remaining hi-only blocks
        lo_last = -1
        if wchunks == 2:
            for bi, blk in enumerate(blocks):
                if any(cs < SQ for (cs, ce, sf) in blk['segs']):
                    lo_last = bi
            if lo_last == len(blocks) - 1:
                lo_last = -1
        # pair up adjacent blocks with identical segment coverage: their
        # probs are summed on the DVE and contribute a single denominator
        # matmul (dnrole: 0 = solo, 1 = defer into next, 2 = paired-with-prev)
        bi = 0
        while bi < len(blocks):
            b0 = blocks[bi]
            if (bi + 1 < len(blocks)
                    and blocks[bi + 1]['segs'] == b0['segs']
                    and all(not sf for (_, _, sf) in blocks[bi + 1]['segs'])):
                b0['dnrole'] = 1
                blocks[bi + 1]['dnrole'] = 2
                bi += 2
            else:
                b0['dnrole'] = 0
                bi += 1
        pairs.append((pidx, pj, wchunks, blocks, lo_last))
        pj += wchunks
        pidx += 1
    return pairs


def _build(classes, nblk, s=S, d=D):
    """Build + compile the per-core SPMD program."""
    nkt = d // P
    n_j = s // SQ
    n_i = s // P
    n_ja = s // CH_A

    nc = bacc.Bacc("TRN2", target_bir_lowering=False, debug=False)
    xT = nc.dram_tensor("xT", (n_ja, P, nkt, CH_A), BF16, kind="ExternalInput")
    wqT = nc.dram_tensor("wqT", (P, nkt, C), BF16, kind="ExternalInput")
    wkT = nc.dram_tensor("wkT", (P, nkt, C), BF16, kind="ExternalInput")
    wvT = nc.dram_tensor("wvT", (P, nkt, C), BF16, kind="ExternalInput")
    woT = nc.dram_tensor("woT", (P, HPG, d), BF16, kind="ExternalInput")
    cosP = nc.dram_tensor("cosP", (HD, s), BF16, kind="ExternalInput")
    sinSw = nc.dram_tensor("sinSw", (HD, s), BF16, kind="ExternalInput")
    mblk = nc.dram_tensor("mblk", (P, nblk, SQ), BF16, kind="ExternalInput")
    onesb = nc.dram_tensor("onesb", (P, 1), BF16, kind="ExternalInput")
    out = nc.dram_tensor("out", (s, d), BF16, kind="ExternalOutput")

    pairs = _pair_blocks(classes, n_j, n_i)

    with tile.TileContext(nc) as tc:
        with tc.tile_pool(name="const", bufs=1) as const:
            cos_t = const.tile([P, s], BF16)
            sin_t = const.tile([P, s], BF16)
            ones_r = const.tile([P, 1], BF16)
            nc.gpsimd.dma_start(ones_r[:], onesb[:])
            mblk_t = const.tile([P, nblk, SQ], BF16)
            qh_sb = const.tile([P, HPG, s], BF16)
            kh_sb = const.tile([P, HPG, s], BF16)
            vh_sb = const.tile([P, n_i, C], BF16)
            attn_t = const.tile([P, HPG, s], BF16)
            wo_t = const.tile([P, HPG, d], BF16)

            # ---------------- Phase A: QKV projections + RoPE ----------------
            with tc.tile_pool(name="wres", bufs=1) as wres, \
                 tc.tile_pool(name="xc", bufs=2) as xcp, \
                 tc.tile_pool(name="ptmp", bufs=4) as ptmp, \
                 tc.tile_pool(name="psW", bufs=HPG, space="PSUM") as psW, \
                 tc.tile_pool(name="psA", bufs=2, space="PSUM") as psA:
                wq_t = wres.tile([P, nkt, C], BF16)
                wk_t = wres.tile([P, nkt, C], BF16)
                wv_t = wres.tile([P, nkt, C], BF16)
                qk4 = nkt // 4
                qk8 = nkt // 8
                # critical-path loads, k-chunked: wq + x chunk 0 only.
                for q4 in range(4):
                    ksl = slice(q4 * qk4, (q4 + 1) * qk4)
                    nc.sync.dma_start(wq_t[:, ksl, :], wqT[:, ksl, :])
                xc_tiles = {}

                def get_xc(j):
                    if j not in xc_tiles:
                        xc_tiles[j] = xcp.tile([P, nkt, CH_A], BF16, tag="xc",
                                               name=f"xc{j}")
                    return xc_tiles[j]

                xc0 = get_xc(0)
                for q4 in range(4):
                    ksl = slice(q4 * qk4, (q4 + 1) * qk4)
                    nc.scalar.dma_start(xc0[:, ksl, :], xT[0][:, ksl, :])
                # gate wk/cos/sin on the last x0 chunk's arrival
                for q4 in range(4):
                    nc.vector.tensor_copy(wk_t[0:1, q4 * qk4, 0:1],
                                          xc0[0:1, nkt - 1, CH_A - 1:CH_A])
                nc.vector.tensor_copy(cos_t[0:1, 0:1],
                                      xc0[0:1, nkt - 1, CH_A - 1:CH_A])
                nc.vector.tensor_copy(sin_t[0:1, 0:1],
                                      xc0[0:1, nkt - 1, CH_A - 1:CH_A])
                for q4 in range(4):
                    ksl = slice(q4 * qk4, (q4 + 1) * qk4)
                    nc.gpsimd.dma_start(wk_t[:, ksl, :], wkT[:, ksl, :])
                nc.sync.dma_start(cos_t[:], cosP[:])
                nc.sync.dma_start(sin_t[:], sinSw[:])

                def rope(ps, dst, ct, sl, j, wt):
                    # RoPE (rotate-half layout):
                    #   out_top = x0*cos - x1*sin ; out_bot = x1*cos + x0*sin
                    # ACT stages PSUM->SBUF bf16 so DVE gets 2x mode
                    pc = ptmp.tile([P, CH_A], BF16, tag="pc")
                    nc.scalar.activation(pc[:], ps[:], COPY)
                    if j == 0 and wt is wq_t and ct == 0:
                        # release wv + x chunk 1 loads (needed ~25us in)
                        for q4 in range(4):
                            nc.vector.tensor_copy(
                                wv_t[0:1, q4 * qk4, 0:1], pc[0:1, 0:1])
                        for q4 in range(4):
                            ksl = slice(q4 * qk4, (q4 + 1) * qk4)
                            nc.gpsimd.dma_start(wv_t[:, ksl, :], wvT[:, ksl, :])
                    if j == 0 and wt is wq_t and ct == 1:
                        xc1 = get_xc(1)
                        nc.vector.tensor_copy(xc1[0:1, 0, 0:1], pc[0:1, 0:1])
                        nc.scalar.dma_start(xc1[:], xT[1])
                    t1 = ptmp.tile([P, CH_A], BF16, tag="t1")
                    nc.vector.tensor_tensor(t1[:], pc[:], cos_t[:, sl], MULT)
                    t2 = ptmp.tile([P, CH_A], BF16, tag="t2")
                    nc.vector.tensor_tensor(
                        t2[0:64, :], pc[64:128, :], sin_t[64:128, sl], MULT)
                    nc.vector.tensor_tensor(
                        t2[64:128, :], pc[0:64, :], sin_t[0:64, sl], MULT)
                    nc.vector.tensor_tensor(dst[:, ct, sl], t1[:], t2[:], ADD)

                for j in range(n_ja):
                    sl = slice(j * CH_A, (j + 1) * CH_A)
                    xc = get_xc(j)
                    if j > 1:
                        nc.scalar.dma_start(xc[:], xT[j])
                    if j == 0:
                        # Q projection in k-waves: wave kc needs only the
                        # kc-th wq/x k-chunk, so the PE starts on the first
                        # 0.5MB of each instead of waiting for the full 2MB.
                        pss = [psW.tile([P, CH_A], F32, tag="ps_w",
                                        name=f"psw{ct}") for ct in range(HPG)]
                        for kc in range(4):
                            for ct in range(HPG):
                                for k in range(kc * qk4, (kc + 1) * qk4):
                                    nc.tensor.matmul(
                                        pss[ct][:],
                                        wq_t[:, k, ct * P:(ct + 1) * P],
                                        xc[:, k, :],
                                        start=(k == 0), stop=(k == nkt - 1),
                                        skip_group_check=True)
                        for ct in range(HPG):
                            rope(pss[ct], qh_sb, ct, sl, j, wq_t)
                        qk_iter = ((wk_t, kh_sb),)
                    else:
                        qk_iter = ((wq_t, qh_sb), (wk_t, kh_sb))
                    for (wt, dst) in qk_iter:
                        for ct in range(HPG):
                            ps = psA.tile([P, CH_A], F32, tag="ps_qk")
                            for k in range(nkt):
                                nc.tensor.matmul(
                                    ps[:], wt[:, k, ct * P:(ct + 1) * P],
                                    xc[:, k, :],
                                    start=(k == 0), stop=(k == nkt - 1))
                            rope(ps, dst, ct, sl, j, wt)
                    for st2 in range(CH_A // P):
                        st = (j * CH_A) // P + st2
                        psv = psA.tile([P, C], F32, tag="ps_v")
                        for k in range(nkt):
                            nc.tensor.matmul(
                                psv[:], xc[:, k, st2 * P:(st2 + 1) * P],
                                wv_t[:, k, :],
                                start=(k == 0), stop=(k == nkt - 1))
                        nc.scalar.activation(vh_sb[:, st, :], psv[:], COPY)
                        if j == 0 and st2 == 0:
                            # release the late constant loads (wo/mblk)
                            nc.vector.tensor_copy(
                                wo_t[0:1, 0, 0:1], vh_sb[0:1, 0, 0:1])
                            nc.vector.tensor_copy(
                                mblk_t[0:1, 0, 0:1], vh_sb[0:1, 0, 0:1])
                            nc.gpsimd.dma_start(wo_t[:], woT[:])
                            nc.gpsimd.dma_start(mblk_t[:], mblk[:])

            # ---------------- Phase B: attention per head ----------------
            # flat pair-block list, software-pipelined with lookahead 1:
            # PE stream is ... sc(n+1), av(n), dn(n) ... so exp(n) on ACT
            # overlaps av(n-1)/dn(n-1)/sc(n+1) on the PE.
            flat = []
            for h in range(HPG):
                for (pidx, base_jq, wchunks, blocks, lo_last) in pairs:
                    nb = len(blocks)
                    for bi, blk in enumerate(blocks):
                        flat.append((h, pidx, base_jq, wchunks, blk,
                                     bi == 0, bi == nb - 1, bi == lo_last))

            with tc.tile_pool(name="pr", bufs=4) as prp, \
                 tc.tile_pool(name="sm", bufs=2) as smp, \
                 tc.tile_pool(name="psS", bufs=2, space="PSUM") as psS, \
                 tc.tile_pool(name="psB", bufs=1, space="PSUM") as psB:
                prs = {}
                gps = {}
                has_lo = set()
                dnpend = {}

                def pieces(cs, ce):
                    # matmul moving operand caps at 512 and PSUM writes must
                    # stay within a bank: split segments at the SQ boundary
                    out = []
                    while cs < ce:
                        pe_ = min(ce, (cs // SQ + 1) * SQ)
                        out.append((cs, pe_))
                        cs = pe_
                    return out

                def emit_score(idx):
                    h, pidx, bjq, wch, blk, first, last_, lo_l = flat[idx]
                    w = wch * SQ
                    sc = psS.tile([P, PW], F32, tag="sc")
                    pr = prp.tile([P, PW], BF16, tag="pr")
                    base = bjq * SQ
                    for (cs, ce, sflag) in blk['segs']:
                        for (ps_, pe_) in pieces(cs, ce):
                            nc.tensor.matmul(
                                sc[:, ps_:pe_],
                                kh_sb[:, h, blk['i'] * P:(blk['i'] + 1) * P],
                                qh_sb[:, h, base + ps_:base + pe_],
                                start=True, stop=True)
                        nc.scalar.activation(
                            pr[:, cs:ce], sc[:, cs:ce], EXP, scale=SCALE)
                    for (off, qoff, cls) in blk['pats']:
                        nc.vector.tensor_tensor(
                            pr[:, off + qoff:off + SQ],
                            pr[:, off + qoff:off + SQ],
                            mblk_t[:, cls, qoff:], MULT)
                    prs[idx] = pr

                def emit_accum(idx):
                    h, pidx, bjq, wch, blk, first, last_, lo_l = flat[idx]
                    w = wch * SQ
                    if first:
                        gps[(h, pidx)] = (
                            psB.tile([P, PW], F32, tag="at",
                                     name=f"at{h}_{pidx}"),
                            psB.tile([1, PW], F32, tag="dn",
                                     name=f"dn{h}_{pidx}"),
                            smp.tile([1, PW], F32, tag="dnsb",
                                     name=f"dnsb{h}_{pidx}"))
                    at_ps, dn_ps, dn_sb = gps[(h, pidx)]
                    pr = prs.pop(idx)
                    i = blk['i']
                    for (cs, ce, sflag) in blk['segs']:
                        for (ps_, pe_) in pieces(cs, ce):
                            nc.tensor.matmul(
                                at_ps[:, ps_:pe_],
                                vh_sb[:, i, h * HD:(h + 1) * HD],
                                pr[:, ps_:pe_],
                                start=sflag, stop=last_, skip_group_check=True)
                    role = blk.get('dnrole', 0)
                    if role == 1:
                        # defer: this block's probs are summed with the next
                        # block's (identical coverage) for one shared dn MM
                        dnpend[(h, pidx)] = (pr, blk['segs'])
                    else:
                        dn_src = pr
                        dn_segs = blk['segs']
                        if role == 2:
                            # start flags come from the earlier block (it may
                            # be the group's initializer)
                            pr0, dn_segs = dnpend.pop((h, pidx))
                            dn_src = prp.tile([P, PW], BF16, tag="prsum")
                            for (cs, ce, sflag) in blk['segs']:
                                nc.vector.tensor_tensor(
                                    dn_src[:, cs:ce], pr0[:, cs:ce],
                                    pr[:, cs:ce], ADD)
                        for (cs, ce, sflag) in dn_segs:
                            for (ps_, pe_) in pieces(cs, ce):
                                nc.tensor.matmul(
                                    dn_ps[:, ps_:pe_], ones_r[:],
                                    dn_src[:, ps_:pe_],
                                    start=sflag, stop=last_,
                                    skip_group_check=True)
                    if lo_l and not last_:
                        # the lo half is final: copy it out now, overlapped
                        # with the remaining hi-only blocks, so the PSUM
                        # buffers recycle fast at the group boundary
                        nc.vector.tensor_copy(
                            attn_t[:, h, bjq * SQ:(bjq + 1) * SQ],
                            at_ps[:, 0:SQ])
                        nc.scalar.activation(dn_sb[:, 0:SQ], dn_ps[:, 0:SQ],
                                             COPY)
                        has_lo.add((h, pidx))
                    if last_:
                        qsl = slice(bjq * SQ, bjq * SQ + w)
                        # hi-half (or full) copies, hoisted ahead of the next
                        # group's queue entries so the buffers free ASAP
                        hi0 = SQ if (h, pidx) in has_lo else 0
                        nc.vector.tensor_copy(
                            attn_t[:, h, bjq * SQ + hi0:bjq * SQ + w],
                            at_ps[:, hi0:w])
                        nc.scalar.activation(dn_sb[:, hi0:w],
                                             dn_ps[:, hi0:w], COPY)
                        # fold + invert denominators; gpsimd broadcast; DVE
                        # normalize (deferred -- not on the PE critical path)
                        nf = w // P
                        dn4 = smp.tile([PW // P, P], F32, tag="dn4")
                        nc.gpsimd.dma_start(dn4[0:nf, :], dn_sb[:, 0:w])
                        rc4 = smp.tile([PW // P, P], F32, tag="rc4")
                        nc.vector.reciprocal(rc4[0:nf, :], dn4[0:nf, :])
                        rc4b = smp.tile([PW // P, P], BF16, tag="rc4b")
                        nc.vector.tensor_copy(rc4b[0:nf, :], rc4[0:nf, :])
                        rc = smp.tile([1, PW], BF16, tag="rc")
                        nc.gpsimd.dma_start(rc[:, 0:w], rc4b[0:nf, :])
                        bc_sb = smp.tile([P, PW], BF16, tag="bcs")
                        nc.gpsimd.partition_broadcast(bc_sb[:, 0:w],
                                                      rc[:, 0:w])
                        nc.vector.tensor_tensor(
                            attn_t[:, h, qsl], attn_t[:, h, qsl],
                            bc_sb[:, 0:w], MULT)

                LA = 1
                nfl = len(flat)
                for idx in range(nfl):
                    emit_score(idx)
                    if idx - LA >= 0:
                        emit_accum(idx - LA)
                for idx in range(nfl - LA, nfl):
                    emit_accum(idx)

            # ---------------- Phase C: output projection ----------------
            with tc.tile_pool(name="og", bufs=2) as ogp, \
                 tc.tile_pool(name="psC", bufs=4, space="PSUM") as psC:
                for st in range(n_i):
                    og = ogp.tile([P, d], BF16, tag="og")
                    for dch in range(d // SQ):
                        po = psC.tile([P, SQ], F32, tag="po")
                        for ct in range(HPG):
                            nc.tensor.matmul(
                                po[:], attn_t[:, ct, st * P:(st + 1) * P],
                                wo_t[:, ct, dch * SQ:(dch + 1) * SQ],
                                start=(ct == 0), stop=(ct == HPG - 1))
                        nc.scalar.activation(
                            og[:, dch * SQ:(dch + 1) * SQ], po[:], COPY)
                        nc.sync.dma_start(
                            out[st * P:(st + 1) * P, dch * SQ:(dch + 1) * SQ],
                            og[:, dch * SQ:(dch + 1) * SQ])

    nc.compile()
    return nc


def _class_key(classes):
    def k(v):
        return v if isinstance(v, str) else tuple(v)
    return tuple(sorted((jk, k(v)) for jk, v in classes.items()))


def _prep_host(inputs):
    """Shard + transpose + bf16-convert the full inputs into 8 per-core maps."""
    x = np.asarray(inputs["x"], np.float32)
    wq = np.asarray(inputs["wq"], np.float32)
    wk = np.asarray(inputs["wk"], np.float32)
    wv = np.asarray(inputs["wv"], np.float32)
    wo = np.asarray(inputs["wo"], np.float32)
    cos = np.asarray(inputs["cos"], np.float32)
    sin = np.asarray(inputs["sin"], np.float32)
    mask = np.asarray(inputs["mask"], np.float32)
    start_p = int(inputs["start_p"])

    s = x.shape[1]
    cos_u = cos[start_p:start_p + s]          # (s, HD/2)
    sin_u = sin[start_p:start_p + s]

    # rotate-half channel permutation within each head: [evens, odds]
    perm = np.concatenate(
        [h * HD + np.concatenate([np.arange(0, HD, 2), np.arange(1, HD, 2)])
         for h in range(H)])

    cosP = np.ascontiguousarray(
        np.concatenate([cos_u.T, cos_u.T], axis=0)).astype(BF)     # (128, s)
    sinSw = np.ascontiguousarray(
        np.concatenate([sin_u.T, -sin_u.T], axis=0)).astype(BF)    # (128, s)

    classes, pats = _classify_mask(mask)
    onesb = np.ones((P, 1), BF)

    in_maps = []
    for b in range(B):
        xTp = _pre_x(np.ascontiguousarray(x[b].T))
        for g in range(GROUPS):
            rows = perm[g * C:(g + 1) * C]
            in_maps.append({
                "xT": xTp,
                "wqT": _pre_w(wq[rows, :].T),
                "wkT": _pre_w(wk[rows, :].T),
                "wvT": _pre_w(wv[g * C:(g + 1) * C, :].T),
                "woT": _pre_w(wo[:, g * C:(g + 1) * C].T),
                "cosP": cosP,
                "sinSw": sinSw,
                "mblk": np.ascontiguousarray(pats.transpose(1, 0, 2)),
                "onesb": onesb,
            })
    return in_maps, classes, pats


def _run(inputs, trace=False):
    in_maps, classes, pats = _prep_host(inputs)
    key = (pats.shape[0], _class_key(classes))
    if key not in _PROGRAM_CACHE:
        _PROGRAM_CACHE[key] = _build(classes, pats.shape[0])
    nc = _PROGRAM_CACHE[key]
    res = bass_utils.run_bass_kernel_spmd(
        nc, in_maps, core_ids=list(range(NCORES)), trace=trace)
    out = np.zeros((B, S, D), np.float32)
    for b in range(B):
        acc = res.results[b * GROUPS]["out"].astype(np.float32).copy()
        for g in range(1, GROUPS):
            acc += res.results[b * GROUPS + g]["out"].astype(np.float32)
        out[b] = acc
    return out, res


def kernel(**inputs):
    out, _ = _run(inputs, trace=False)
    return out
